# revision 58
# baseline (speedup 1.0000x reference)
"""Trainium2 Bass kernel for nn_BidirectionalMamba.

Self-contained: hardcodes shapes from the problem spec.

Sharding (8 cores): core = dir*4 + batch*2 + dhalf
  - dir   in {0,1}: forward chain (blocks 0,1) / backward chain (blocks 2,3,
            fed time-reversed input, un-reversed on host)
  - batch in {0,1}
  - dhalf in {0,1}: each core owns 256 of 512 d_inner channels for the scan
            path; stage-0 (LN/in-proj/conv/rank projections) is replicated
            within the pair.

Scan layout: per-(group, state) tiles [128 channels, T] with the sequence in
the free dim.  dA_n = exp(A[:,n] * delta) comes straight from the scalar
engine (per-partition scale vector), dBx/hC are wide bf16 multiplies on DVE
against B/C rows broadcast to 128 partitions via DMA (DRAM bounce with a
stride-0 source), and the recurrence itself is tensor_tensor_scan on the
Pool engine.  y accumulates over the 16 states with wide bf16 adds.

Cross-core: only the block0->block1 boundary needs the d-half pair's
out-projection partials; that is ONE AllGather per chain (bf16, two
half-sequence pieces so it pipelines with the scan).  The block1 output is
never exchanged: each core multiplies (alpha*residual + own partial) by the
full per-direction merge weight and the host sums the four per-core merge
partials per batch.
"""

import numpy as np
import ml_dtypes

BF = ml_dtypes.bfloat16

B_, S_, DM, DI, N_, R_, K_ = 2, 2048, 256, 512, 16, 32, 4
DH = DI // 2            # 256 own channels per core
NB = 2                  # blocks per chain (per core)
TCk = 512               # time chunk (psum-sized)
NCH = S_ // TCk         # 4
HW_ = S_ // 2           # half-sequence width for wide ops
NXT = DM // 128         # 2 x-tiles
NDT = DI // 128         # 4 full-d tiles
NG = DH // 128          # 2 own 128-channel groups

_BUILD_CACHE = {}
import os
_USE_COLLECTIVE = os.environ.get("NO_CC", "") == ""
# how many of the 32 (g,n) scans per block run on DVE instead of Pool
_SCAN_DVE = int(os.environ.get("SCAN_DVE", "32"))
# how many of the 16 n's get dA built by DVE squaring instead of Act exp
_POW_DVE = int(os.environ.get("POW_DVE", "0"))


# ---------------------------------------------------------------- host prep

def _host_inputs(inputs):
    x = np.ascontiguousarray(inputs['x'], dtype=np.float32)        # [B,S,DM]
    in_maps = []
    for core in range(8):
        d = core // 4          # dir
        b = (core // 2) % 2    # batch
        dh = core % 2          # d-half
        # channel permutation: own 256 first
        perm = np.concatenate([np.arange(dh * DH, (dh + 1) * DH),
                               np.arange((1 - dh) * DH, (2 - dh) * DH)])
        own = slice(dh * DH, (dh + 1) * DH)
        m = {}
        xb = x[b] if d == 0 else x[b, ::-1]
        m['x_in'] = np.ascontiguousarray(xb.T).reshape(NXT, 128, S_)

        for i in range(NB):
            g = 2 * d + i      # global block index
            ln_g = inputs['ln_g'][g].astype(np.float32)
            ln_b = inputs['ln_b'][g].astype(np.float32)
            in_w = inputs['in_w'][g].astype(np.float32)            # [DM, 2DI]
            w_scaled = ln_g[:, None] * in_w
            bvec = ln_b @ in_w                                     # [2DI]
            # columns: xs (512, permuted own-first) + z own (256)
            cols = np.concatenate([perm, DI + perm[:DH]])
            wsel = w_scaled[:, cols]                               # [256, 768]
            bsel = bvec[cols]                                      # [768]
            m[f'inw_{i}'] = np.ascontiguousarray(wsel.reshape(2, 128, 768)).astype(BF)
            # z bias (own 256)
            m[f'zb_{i}'] = np.ascontiguousarray(
                bsel[DI:DI + DH].reshape(NG, 128, 1))
            cw = inputs['conv_w'][g].astype(np.float32)[perm]      # [DI, K]
            cwd = np.zeros((NDT, 128, K_ * 128), np.float32)
            for j in range(NDT):
                for k in range(K_):
                    cwd[j, np.arange(128), k * 128 + np.arange(128)] = \
                        cw[j * 128:(j + 1) * 128, k]
            m[f'convw_{i}'] = cwd.astype(BF)
            # conv bias with the xs-bias folded in:
            # conv(xs + beta) = conv(xs) + beta * sum_k w   (interior exact;
            # beta = ln_b@in_w is zero for this problem anyway)
            beta = bsel[:DI]
            cb = inputs['conv_b'][g].astype(np.float32)[perm] \
                + beta * cw.sum(axis=1)
            m[f'convb_{i}'] = np.ascontiguousarray(cb.reshape(NDT, 128, 1))
            xp = np.concatenate([inputs['xd_w'][g], inputs['xB_w'][g],
                                 inputs['xC_w'][g]], axis=1)[perm]  # [DI, 64]
            m[f'xproj_{i}'] = np.ascontiguousarray(
                xp.astype(np.float32).reshape(NDT, 128, R_ + 2 * N_)).astype(BF)
            m[f'dtpw_{i}'] = np.ascontiguousarray(
                inputs['dtp_w'][g].astype(np.float32)[:, own]).astype(BF)
            m[f'dtpb_{i}'] = np.ascontiguousarray(
                inputs['dtp_b'][g][own].astype(np.float32).reshape(NG, 128, 1))
            A = -np.exp(inputs['A_log'][g].astype(np.float64)).astype(np.float32)
            m[f'acol_{i}'] = np.ascontiguousarray(
                A[own].reshape(NG, 128, N_))                        # [2,128,16]
            m[f'dvec_{i}'] = np.ascontiguousarray(
                inputs['D'][g][own].astype(np.float32).reshape(NG, 128, 1))
            ow = inputs['out_w'][g].astype(np.float32)[own]         # [256, 256]
            m[f'outw_{i}'] = np.ascontiguousarray(ow.reshape(2, 128, DM)).astype(BF)

        mw = inputs['merge_w'].astype(np.float32)                   # [512, 256]
        m['mergew'] = np.ascontiguousarray(
            mw[d * 256:(d + 1) * 256, :].reshape(2, 128, DM)).astype(BF)
        av = np.full((128, 1), 1.0 if dh == 0 else 0.0, np.float32)
        m['alphav'] = av
        m['ones128'] = np.full((128, 1), 1.0 / DM, np.float32)
        cb2 = np.zeros((128, 2), np.float32)
        cb2[:, 0] = 1e-5
        cb2[:, 1] = 1.0
        m['cbias'] = cb2
        m['ones1'] = np.ones((1, 128), BF)
        m['ident'] = np.eye(128, dtype=np.float32).astype(BF)
        in_maps.append(m)
    return in_maps


# ---------------------------------------------------------------- device

def _build_program():
    from contextlib import ExitStack
    import concourse.bass as bass
    import concourse.tile as tile
    from concourse import bacc, mybir

    f32 = mybir.dt.float32

    # Route Exp and Ln to the combined natural_log_exp table set so the
    # softplus (Exp then Ln) and dA-exp phases share one resident table.
    from concourse import hw_specs as _hw
    _orig_tables = _hw.get_activation_tables
    def _filtered_tables(arch):
        t = dict(_orig_tables(arch))
        for drop in ('exp_and_others', 'exp_and_friends', 'natural_log'):
            if drop in t:
                t[drop] = frozenset()   # keep index alignment, never matched
        return t
    bacc.get_activation_tables = _filtered_tables

    nc = bacc.Bacc("TRN2", target_bir_lowering=False, debug=False,
                   num_devices=8)

    f32r = mybir.dt.float32r

    bf16 = mybir.dt.bfloat16

    def din(name, shape, dt=None):
        return nc.dram_tensor(name, list(shape), dt or f32,
                              kind="ExternalInput").ap()

    W = {'x_in': din('x_in', (NXT, 128, S_), f32r)}
    for i in range(NB):
        W[f'inw_{i}'] = din(f'inw_{i}', (2, 128, 768), bf16)
        W[f'zb_{i}'] = din(f'zb_{i}', (NG, 128, 1))
        W[f'convw_{i}'] = din(f'convw_{i}', (NDT, 128, K_ * 128), bf16)
        W[f'convb_{i}'] = din(f'convb_{i}', (NDT, 128, 1))
        W[f'xproj_{i}'] = din(f'xproj_{i}', (NDT, 128, R_ + 2 * N_), bf16)
        W[f'dtpw_{i}'] = din(f'dtpw_{i}', (R_, DH), bf16)
        W[f'dtpb_{i}'] = din(f'dtpb_{i}', (NG, 128, 1))
        W[f'acol_{i}'] = din(f'acol_{i}', (NG, 128, N_))
        W[f'dvec_{i}'] = din(f'dvec_{i}', (NG, 128, 1))
        W[f'outw_{i}'] = din(f'outw_{i}', (2, 128, DM), bf16)
    W['mergew'] = din('mergew', (2, 128, DM), bf16)
    W['alphav'] = din('alphav', (128, 1))
    W['ones128'] = din('ones128', (128, 1), f32r)
    W['cbias'] = din('cbias', (128, 2))
    W['ones1'] = din('ones1', (1, 128), bf16)
    W['ident'] = din('ident', (128, 128), bf16)

    outp = nc.dram_tensor('outp', [DM, S_], f32, kind="ExternalOutput").ap()

    with tile.TileContext(nc) as tc:
        with ExitStack() as ctx:
            _emit(ctx, nc, tc, bass, mybir, f32, W, outp)
    nc.compile()
    return nc


def _emit(ctx, nc, tc, bass, mybir, f32, W, outp):
    EX = ctx.enter_context
    AF = mybir.ActivationFunctionType
    OPa, OPm = mybir.AluOpType.add, mybir.AluOpType.mult
    f32r = mybir.dt.float32r
    bf16 = mybir.dt.bfloat16
    ts = bass.ts

    # ---- pools
    wpool = EX(tc.tile_pool(name="wconst", bufs=1))
    wblk = EX(tc.tile_pool(name="wblk", bufs=1))
    xio = EX(tc.tile_pool(name="xio", bufs=2))          # xT tiles f32r
    lnp = EX(tc.tile_pool(name="lnp", bufs=1))
    stg = EX(tc.tile_pool(name="stg", bufs=2))          # xn / tmp chunks
    xsp = EX(tc.tile_pool(name="xsp", bufs=2))          # xs halo chunks
    xcp = EX(tc.tile_pool(name="xcp", bufs=2))          # xc chunks bf16
    rkp = EX(tc.tile_pool(name="rkp", bufs=2))          # rank rows chunks
    widep = EX(tc.tile_pool(name="widep", bufs=1))      # per-block wide tiles
    bbp = EX(tc.tile_pool(name="bbp", bufs=3))          # Bb/Cb bcast tiles
    dap = EX(tc.tile_pool(name="dap", bufs=5))          # dA (deep, batches Act)
    scp = EX(tc.tile_pool(name="scp", bufs=3))          # dBx/hC
    hp = EX(tc.tile_pool(name="hp", bufs=3))            # h (shared tag)
    cyp = EX(tc.tile_pool(name="cyp", bufs=1))          # half-boundary carries
    pop = EX(tc.tile_pool(name="pop", bufs=2))

    ps_stage = EX(tc.tile_pool(name="ps_stage", bufs=3, space="PSUM"))
    ps_out = EX(tc.tile_pool(name="ps_out", bufs=1, space="PSUM"))
    ps_acc = EX(tc.tile_pool(name="ps_acc", bufs=1, space="PSUM"))
    dram = EX(tc.tile_pool(name="dram", bufs=2, space="DRAM"))

    # ---- constants
    def cw(name, shape, dt=None):
        t = wpool.tile(list(shape), dt or f32, tag=name)
        nc.sync.dma_start(t[:], W[name][:])
        return t

    ones128 = cw('ones128', (128, 1), f32r)
    cbias = cw('cbias', (128, 2))
    ones1 = cw('ones1', (1, 128), bf16)
    ident = cw('ident', (128, 128), bf16)
    alphav = cw('alphav', (128, 1))
    mergew = [wpool.tile([128, DM], bf16, tag=f"mgw{j}", name=f"mgw{j}")
              for j in range(2)]
    for j in range(2):
        nc.sync.dma_start(mergew[j][:], W['mergew'][j])

    # ---- x input (per half-sequence tiles)
    xT = [[xio.tile([128, HW_], f32r, tag=f"xT{j}{hh}", name=f"xTin{j}{hh}")
           for hh in range(2)] for j in range(NXT)]
    for j in range(NXT):
        for hh in range(2):
            nc.sync.dma_start(xT[j][hh][:],
                              W['x_in'][j, :, hh * HW_:(hh + 1) * HW_])

    replica_groups = [[0, 1], [2, 3], [4, 5], [6, 7]]

    from concourse.tile import add_dep_helper
    first_rb = None

    for blk in range(NB):
        last = blk == NB - 1
        # ---- per-block weights (tags read late in the previous block get
        # bufs=2 so this block's loads are not serialized behind them)
        w = {}
        for nm, cnt, shp in [('inw', 2, (128, 768)), ('zb', NG, (128, 1)),
                             ('convw', NDT, (128, K_ * 128)),
                             ('convb', NDT, (128, 1)),
                             ('xproj', NDT, (128, R_ + 2 * N_)),
                             ('dtpb', NG, (128, 1)), ('acol', NG, (128, N_)),
                             ('dvec', NG, (128, 1)), ('outw', 2, (128, DM))]:
            wdt = bf16 if nm in ('inw', 'xproj', 'outw', 'convw') else f32
            wbufs = 2 if nm in ('acol', 'dvec', 'outw') else 1
            tl = []
            for j in range(cnt):
                t = wblk.tile(list(shp), wdt, tag=f"{nm}{j}", bufs=wbufs)
                wdma = nc.sync.dma_start(t[:], W[f'{nm}_{blk}'][j])
                if blk == 1:
                    tc.chain_iter_dep("rbgate", wdma.ins)
                tl.append(t)
            w[nm] = tl
        w['dtpw'] = wblk.tile([R_, DH], bf16, tag="dtpw", name=f"dtpw{blk}")
        last_wdma = nc.sync.dma_start(w['dtpw'][:], W[f'dtpw_{blk}'][:])

        pending_h1 = None
        if blk == 1:
            # deferred h1 readback of the block0 exchange: the DMAs go out
            # now (ordered after this block's weight loads in the SP queue);
            # the residual adds are emitted later, right before the stage
            # needs the h1 residual, so they don't block the DVE queue.
            bounce_out0, xT_prev = deferred_rb
            tc.chain_iter_dep("rbgate", last_wdma.ins)
            pending_h1 = []
            for m in range(NXT):
                p0 = pop.tile([128, HW_], bf16, tag="peer0", bufs=2,
                              name=f"p0_1_{m}")
                p1 = pop.tile([128, HW_], bf16, tag="peer1", bufs=2,
                              name=f"p1_1_{m}")
                rb = nc.sync.dma_start(
                    p0[:], bounce_out0[1][128 * m:128 * (m + 1), :])
                tc.chain_iter_dep("rbgate", rb.ins)
                rb2 = nc.sync.dma_start(
                    p1[:], bounce_out0[1][DM + 128 * m:DM + 128 * (m + 1), :])
                tc.chain_iter_dep("rbgate", rb2.ins)
                pending_h1.append((p0, p1, m))

        # wide per-block tiles (per half h)
        delw = [[widep.tile([128, HW_], bf16, tag=f"delw{g}{h}",
                            name=f"delw{blk}_{g}{h}") for h in range(2)]
                for g in range(NG)]
        dxw = [[widep.tile([128, HW_], bf16, tag=f"dxw{g}{h}",
                           name=f"dxw{blk}_{g}{h}") for h in range(2)]
               for g in range(NG)]
        xcow = [[widep.tile([128, HW_], bf16, tag=f"xcow{g}{h}",
                            name=f"xcow{blk}_{g}{h}") for h in range(2)]
                for g in range(NG)]
        zw = [[widep.tile([128, HW_], bf16, tag=f"zw{g}{h}",
                          name=f"zw{blk}_{g}{h}") for h in range(2)]
              for g in range(NG)]
        bcBC = [widep.tile([R_, HW_], bf16, tag=f"bcBC{h}",
                           name=f"bcBC{blk}_{h}") for h in range(2)]

        # ============================== stage: chunk loop
        xs_prev = [None] * NDT
        xcw_t = [None] * NCH
        for c in range(NCH):
            cs = ts(c, TCk)
            h = c // 2
            hs = ts(c % 2, TCk)    # slice within the half-wide tiles
            if c == 2 and pending_h1 is not None:
                for p0, p1, m in pending_h1:
                    psum12 = stg.tile([128, HW_], f32, tag="psum12", bufs=2,
                                      name=f"ps12_1_{m}")
                    nc.gpsimd.tensor_add(psum12[:], p0[:], p1[:])
                    nc.gpsimd.tensor_add(xT[m][1][:], psum12[:],
                                         xT_prev[m][1][:])
                pending_h1 = None
            # LN stats for this chunk
            st = ps_stage.tile([128, TCk], f32, tag="ps", name=f"st{c}")
            st2 = ps_stage.tile([128, TCk], f32, tag="ps", name=f"st2{c}")
            sq = [stg.tile([128, TCk], f32r, tag="sq", bufs=2, name=f"sq{j}_{c}")
                  for j in range(NXT)]
            for j in range(NXT):
                nc.gpsimd.tensor_mul(sq[j][:], xT[j][h][:, hs], xT[j][h][:, hs])
            for j in range(NXT):
                nc.tensor.matmul(st[0:1, :], ones128[:], xT[j][h][:, hs],
                                 start=(j == 0), stop=(j == NXT - 1))
            for j in range(NXT):
                nc.tensor.matmul(st2[0:1, :], ones128[:], sq[j][:],
                                 start=(j == 0), stop=(j == NXT - 1))
            sumx = lnp.tile([1, TCk], bf16, tag=f"sumx{c % 2}", name=f"sumx{c}")
            rstd = lnp.tile([1, TCk], bf16, tag=f"rstd{c % 2}", name=f"rstd{c}")
            nc.scalar.copy(sumx[:], st[0:1, :])
            var = lnp.tile([1, TCk], f32, tag="var", bufs=2, name=f"var{c}")
            mm = lnp.tile([1, TCk], f32, tag="mm", bufs=2, name=f"mm{c}")
            nc.vector.tensor_mul(mm[:], sumx[:], sumx[:])
            nc.vector.tensor_sub(var[:], st2[0:1, :], mm[:])
            nc.scalar.activation(var[:], var[:], AF.Ln, bias=cbias[0:1, 0:1])
            nc.scalar.activation(rstd[:], var[:], AF.Exp, scale=-0.5)
            mrep = ps_stage.tile([128, TCk], f32, tag="ps", name=f"mrep{c}")
            rrep = ps_stage.tile([128, TCk], f32, tag="ps", name=f"rrep{c}")
            nc.tensor.matmul(mrep[:], ones1[:], sumx[:],
                             start=True, stop=True)
            nc.tensor.matmul(rrep[:], ones1[:], rstd[:],
                             start=True, stop=True)
            mrepS = stg.tile([128, TCk], f32, tag="mrepS", bufs=1)
            rrepS = stg.tile([128, TCk], f32, tag="rrepS", bufs=1)
            nc.vector.tensor_copy(mrepS[:], mrep[:])
            nc.vector.tensor_copy(rrepS[:], rrep[:])
            xn = []
            for j in range(NXT):
                t = stg.tile([128, TCk], bf16, tag=f"xn{j}")
                tf = stg.tile([128, TCk], f32, tag="xnf", name=f"xnf{c}_{j}")
                nc.gpsimd.tensor_sub(tf[:], xT[j][h][:, hs], mrepS[:])
                nc.gpsimd.tensor_mul(t[:], tf[:], rrepS[:])
                xn.append(t)
            # in-proj
            xc_t = [None] * NDT
            for m in range(6):
                ps = ps_stage.tile([128, TCk], f32, tag="ps", name=f"pi{c}_{m}")
                for k in range(2):
                    nc.tensor.matmul(ps[:], w['inw'][k][:, ts(m, 128)],
                                     xn[k][:], start=(k == 0), stop=(k == 1))
                if m < NDT:
                    xt = xsp.tile([128, TCk + 4], bf16, tag=f"xs{m}")
                    nc.scalar.copy(xt[:, 4:], ps[:])
                    if c == 0:
                        nc.vector.memset(xt[:, 0:4].bitcast(mybir.dt.uint16), 0)
                    else:
                        nc.vector.tensor_copy(
                            xt[:, 1:4], xs_prev[m][:, TCk + 1:TCk + 4])
                    psc = ps_stage.tile([128, TCk], f32, tag="ps",
                                        name=f"psc{c}_{m}")
                    for sh in range(K_):
                        nc.tensor.matmul(
                            psc[:], w['convw'][m][:, ts(3 - sh, 128)],
                            xt[:, 4 - sh:TCk + 4 - sh],
                            start=(sh == 0), stop=(sh == K_ - 1))
                    xc = xcp.tile([128, TCk], bf16, tag=f"xc{m}")
                    nc.scalar.activation(xc[:], psc[:], AF.Silu,
                                         bias=w['convb'][m][:])
                    xs_prev[m] = xt
                    xc_t[m] = xc
                    if m < NG:
                        nc.vector.tensor_copy(xcow[m][h][:, hs], xc[:])
                else:
                    g = m - NDT
                    nc.scalar.activation(zw[g][h][:, hs], ps[:], AF.Silu,
                                         bias=w['zb'][g][:])
            # rank projections: one [64, TCk] psum
            psr = ps_stage.tile([128, TCk], f32, tag="ps", name=f"prr{c}")
            for k in range(NDT):
                nc.tensor.matmul(psr[0:R_ + 2 * N_, :], w['xproj'][k][:],
                                 xc_t[k][:], start=(k == 0),
                                 stop=(k == NDT - 1))
            xcw = rkp.tile([R_, TCk], bf16, tag="xcw")
            nc.vector.tensor_copy(xcw[:], psr[0:R_, :])
            nc.vector.tensor_copy(bcBC[h][:, hs], psr[R_:R_ + R_, :])
            xcw_t[c] = xcw
            # delta = softplus(dtpw @ xcw + dtpb), batched per half so the
            # Exp/Ln table is loaded once after the half's Silu run
            if c % 2 == 1:
                for c2 in range(2):
                    hs2 = ts(c2, TCk)
                    for g in range(NG):
                        psd = ps_stage.tile([128, TCk], f32, tag="ps",
                                            name=f"pd{c}_{c2}_{g}")
                        nc.tensor.matmul(psd[:],
                                         w['dtpw'][:, ts(g, 128)],
                                         xcw_t[2 * h + c2][:],
                                         start=True, stop=True)
                        et = stg.tile([128, TCk], f32, tag="sptmp")
                        nc.scalar.activation(et[:], psd[:], AF.Exp,
                                             bias=w['dtpb'][g][:])
                        nc.scalar.activation(delw[g][h][:, hs2], et[:],
                                             AF.Ln, bias=cbias[:, 1:2])

        # ============================== scan phase, per half
        dbc = dram.tile([R_, S_], bf16, tag="dbc")
        accs = [[None] * NG, [None] * NG]
        carry = {}
        bounce_h1_dmas = []
        if blk == 0:
            partial = [[pop.tile([128, HW_], bf16, tag=f"pm{m}{hh}", bufs=1,
                                 name=f"par{m}{hh}") for hh in range(2)]
                       for m in range(NXT)]
            bounce_in = [dram.tile([DM, HW_], bf16, tag=f"bin{hh}",
                                   name=f"bin{hh}") for hh in range(2)]
            bounce_out = [dram.tile([2 * DM, HW_], bf16, tag=f"bout{hh}",
                                    name=f"bout{hh}") for hh in range(2)]
            xT_next = [[xio.tile([128, HW_], f32r, tag=f"xT{j}{hh}",
                                 name=f"xTn{j}{hh}") for hh in range(2)]
                       for j in range(NXT)]
        else:
            preM = [[pop.tile([128, HW_], bf16, tag=f"pm{m}{hh}", bufs=1,
                              name=f"preM{m}{hh}") for hh in range(2)]
                    for m in range(NXT)]
        for h in range(2):
            hsl = slice(h * HW_, (h + 1) * HW_)
            nc.sync.dma_start(dbc[:, hsl], bcBC[h][:])
            # dx = delta * xc (own), wide
            for g in range(NG):
                nc.vector.tensor_mul(dxw[g][h][:], delw[g][h][:],
                                     xcow[g][h][:])
            for n in range(N_):
                Bb = bbp.tile([128, HW_], bf16, tag="Bb")
                nc.sync.dma_start(
                    Bb[:], dbc[n:n + 1, hsl].broadcast_to((128, HW_)))
                Cb = bbp.tile([128, HW_], bf16, tag="Cb")
                cb_dma = nc.sync.dma_start(
                    Cb[:], dbc[N_ + n:N_ + n + 1, hsl].broadcast_to((128, HW_)))
                if blk == 0 and h == 1 and n == N_ - 1:
                    tc.chain_iter_dep("rbgate", cb_dma.ins)
                for g in range(NG):
                    k = n * NG + g
                    dA = dap.tile([128, HW_], bf16, tag="dA")
                    nc.scalar.activation(dA[:], delw[g][h][:], AF.Exp,
                                         scale=w['acol'][g][:, n:n + 1])
                    dBx = scp.tile([128, HW_], bf16, tag="dBx")
                    beng = nc.gpsimd if n >= 11 else nc.vector
                    beng.tensor_mul(dBx[:], dxw[g][h][:], Bb[:])
                    ht = hp.tile([128, HW_], bf16, tag="h")
                    eng = nc.vector if (k % 32) < _SCAN_DVE else nc.gpsimd
                    eng.tensor_tensor_scan(
                        ht[:], dA[:], dBx[:],
                        0.0 if h == 0 else carry[(g, n)][:],
                        op0=OPm, op1=OPa)
                    if h == 0:
                        cy = cyp.tile([128, 1], bf16, tag=f"cy{g}_{n}")
                        nc.gpsimd.tensor_copy(cy[:], ht[:, HW_ - 1:HW_])
                        carry[(g, n)] = cy
                    hC = scp.tile([128, HW_], bf16, tag="hC")
                    heng = nc.gpsimd if n < 8 else nc.vector
                    heng.tensor_mul(hC[:], ht[:], Cb[:])
                    for c2 in range(2):
                        if n == 0 and g == 0 and c2 == 0:
                            pacc = [[ps_acc.tile([128, TCk], f32,
                                                 tag=f"acc{gg}{cc}",
                                                 name=f"pac{blk}{h}_{gg}{cc}")
                                     for cc in range(2)] for gg in range(NG)]
                        nc.tensor.matmul(pacc[g][c2][:], ident[:],
                                         hC[:, ts(c2, TCk)],
                                         start=(n == 0), stop=(n == N_ - 1))
            for g in range(NG):
                acc = widep.tile([128, HW_], bf16, tag=f"acc{g}{h}",
                                 name=f"acc{blk}_{g}{h}")
                accs[h][g] = acc
                for c2 in range(2):
                    nc.scalar.copy(acc[:, ts(c2, TCk)], pacc[g][c2][:])

            # ---- post + out-proj for this half
            gz = []
            for g in range(NG):
                yg = accs[h][g]
                nc.vector.scalar_tensor_tensor(
                    yg[:], xcow[g][h][:], w['dvec'][g][:], yg[:],
                    OPm, OPa)
                nc.vector.tensor_mul(yg[:], yg[:], zw[g][h][:])
                gz.append(yg)
            for c2 in range(2):
                c = 2 * h + c2
                cs = ts(c, TCk)
                hs = ts(c2, TCk)
                for m in range(NXT):
                    pso = ps_out.tile([128, TCk], f32, tag="pso",
                                      name=f"pso{blk}_{c}_{m}")
                    for g in range(NG):
                        nc.tensor.matmul(pso[:], w['outw'][g][:, ts(m, 128)],
                                         gz[g][:, hs], start=(g == 0),
                                         stop=(g == 1))
                    if blk == 0:
                        nc.scalar.copy(partial[m][h][:, hs], pso[:])
                    else:
                        nc.vector.scalar_tensor_tensor(
                            preM[m][h][:, hs], xT[m][h][:, hs], alphav[:],
                            pso[:], OPm, OPa)
            if blk == 0:
                # issue this half's pair exchange (readback deferred so the
                # DMA queue keeps feeding the other half's scan)
                for m in range(NXT):
                    bi_dma = nc.sync.dma_start(
                        bounce_in[h][128 * m:128 * (m + 1), :],
                        partial[m][h][:])
                    if h == 1:
                        bounce_h1_dmas.append(bi_dma)
                if _USE_COLLECTIVE:
                    nc.gpsimd.collective_compute(
                        "AllGather", mybir.AluOpType.bypass,
                        replica_groups=replica_groups,
                        ins=[bounce_in[h][:].opt()],
                        outs=[bounce_out[h][:].opt()])
                else:
                    nc.sync.dma_start(bounce_out[h][0:DM, :], bounce_in[h][:])
                    nc.sync.dma_start(bounce_out[h][DM:2 * DM, :],
                                      bounce_in[h][:])
            else:
                # merge partial for this half
                for c2 in range(2):
                    c = 2 * h + c2
                    cs = ts(c, TCk)
                    hs = ts(c2, TCk)
                    for m in range(NXT):
                        ps = ps_out.tile([128, TCk], f32, tag="pso",
                                         name=f"mg{c}_{m}")
                        for j in range(2):
                            nc.tensor.matmul(ps[:], mergew[j][:, ts(m, 128)],
                                             preM[j][h][:, hs],
                                             start=(j == 0), stop=(j == 1))
                        op = pop.tile([128, TCk], f32, tag="op", bufs=2)
                        nc.scalar.copy(op[:], ps[:])
                        nc.sync.dma_start(outp[128 * m:128 * (m + 1), cs],
                                          op[:])
        if blk == 0:
            # h0 readback + residual adds, gated (chain dep) behind the h1
            # broadcast feed so its AllGather wait cannot starve the second
            # half's scan.  The h1 readback is deferred into block 1.
            for m in range(NXT):
                p0 = pop.tile([128, HW_], bf16, tag="peer0", bufs=2,
                              name=f"p0_0_{m}")
                p1 = pop.tile([128, HW_], bf16, tag="peer1", bufs=2,
                              name=f"p1_0_{m}")
                rb = nc.sync.dma_start(p0[:],
                                       bounce_out[0][128 * m:128 * (m + 1), :])
                tc.chain_iter_dep("rbgate", rb.ins)
                rb2 = nc.sync.dma_start(
                    p1[:], bounce_out[0][DM + 128 * m:DM + 128 * (m + 1), :])
                tc.chain_iter_dep("rbgate", rb2.ins)
                psum12 = stg.tile([128, HW_], f32, tag="psum12", bufs=2,
                                  name=f"ps12_0_{m}")
                nc.gpsimd.tensor_add(psum12[:], p0[:], p1[:])
                nc.gpsimd.tensor_add(xT_next[m][0][:], psum12[:],
                                     xT[m][0][:])
            deferred_rb = (bounce_out, xT)
            xT = xT_next


# ---------------------------------------------------------------- entry

def kernel(**inputs) -> np.ndarray:
    from concourse.bass_utils import run_bass_kernel_spmd

    if 'nc' not in _BUILD_CACHE:
        _BUILD_CACHE['nc'] = _build_program()
    nc = _BUILD_CACHE['nc']

    in_maps = _host_inputs(inputs)
    res = run_bass_kernel_spmd(nc, in_maps, core_ids=list(range(8)))
    _BUILD_CACHE['last_res'] = res
    parts = [r['outp'] for r in res.results]   # [256, 2048] each

    out = np.zeros((B_, S_, DM), np.float32)
    for b in range(B_):
        acc = np.zeros((DM, S_), np.float32)
        for dh in range(2):
            acc += parts[0 * 4 + b * 2 + dh]               # fwd
            acc += parts[1 * 4 + b * 2 + dh][:, ::-1]      # bwd
        out[b] = acc.T
    return out


# revision 66
# speedup vs baseline: 1.0085x; 1.0085x over previous
"""Trainium2 Bass kernel for nn_BidirectionalMamba.

Self-contained: hardcodes shapes from the problem spec.

Sharding (8 cores): core = dir*4 + batch*2 + dhalf
  - dir   in {0,1}: forward chain (blocks 0,1) / backward chain (blocks 2,3,
            fed time-reversed input, un-reversed on host)
  - batch in {0,1}
  - dhalf in {0,1}: each core owns 256 of 512 d_inner channels for the scan
            path; stage-0 (LN/in-proj/conv/rank projections) is replicated
            within the pair.

Scan layout: per-(group, state) tiles [128 channels, T] with the sequence in
the free dim.  dA_n = exp(A[:,n] * delta) comes straight from the scalar
engine (per-partition scale vector), dBx/hC are wide bf16 multiplies on DVE
against B/C rows broadcast to 128 partitions via DMA (DRAM bounce with a
stride-0 source), and the recurrence itself is tensor_tensor_scan on the
Pool engine.  y accumulates over the 16 states with wide bf16 adds.

Cross-core: only the block0->block1 boundary needs the d-half pair's
out-projection partials; that is ONE AllGather per chain (bf16, two
half-sequence pieces so it pipelines with the scan).  The block1 output is
never exchanged: each core multiplies (alpha*residual + own partial) by the
full per-direction merge weight and the host sums the four per-core merge
partials per batch.
"""

import numpy as np
import ml_dtypes

BF = ml_dtypes.bfloat16

B_, S_, DM, DI, N_, R_, K_ = 2, 2048, 256, 512, 16, 32, 4
DH = DI // 2            # 256 own channels per core
NB = 2                  # blocks per chain (per core)
TCk = 512               # time chunk (psum-sized)
NCH = S_ // TCk         # 4
HW_ = S_ // 2           # half-sequence width for wide ops
NXT = DM // 128         # 2 x-tiles
NDT = DI // 128         # 4 full-d tiles
NG = DH // 128          # 2 own 128-channel groups

_BUILD_CACHE = {}
import os
_USE_COLLECTIVE = os.environ.get("NO_CC", "") == ""
# how many of the 32 (g,n) scans per block run on DVE instead of Pool
_SCAN_DVE = int(os.environ.get("SCAN_DVE", "32"))
# how many of the 16 n's get dA built by DVE squaring instead of Act exp
_POW_DVE = int(os.environ.get("POW_DVE", "0"))


# ---------------------------------------------------------------- host prep

def _host_inputs(inputs):
    x = np.ascontiguousarray(inputs['x'], dtype=np.float32)        # [B,S,DM]
    in_maps = []
    for core in range(8):
        d = core // 4          # dir
        b = (core // 2) % 2    # batch
        dh = core % 2          # d-half
        # channel permutation: own 256 first
        perm = np.concatenate([np.arange(dh * DH, (dh + 1) * DH),
                               np.arange((1 - dh) * DH, (2 - dh) * DH)])
        own = slice(dh * DH, (dh + 1) * DH)
        m = {}
        xb = x[b] if d == 0 else x[b, ::-1]
        m['x_in'] = np.ascontiguousarray(xb.T).reshape(NXT, 128, S_)

        for i in range(NB):
            g = 2 * d + i      # global block index
            ln_g = inputs['ln_g'][g].astype(np.float32)
            ln_b = inputs['ln_b'][g].astype(np.float32)
            in_w = inputs['in_w'][g].astype(np.float32)            # [DM, 2DI]
            w_scaled = ln_g[:, None] * in_w
            bvec = ln_b @ in_w                                     # [2DI]
            # columns: xs (512, permuted own-first) + z own (256)
            cols = np.concatenate([perm, DI + perm[:DH]])
            wsel = w_scaled[:, cols]                               # [256, 768]
            bsel = bvec[cols]                                      # [768]
            m[f'inw_{i}'] = np.ascontiguousarray(wsel.reshape(2, 128, 768)).astype(BF)
            # z bias (own 256)
            m[f'zb_{i}'] = np.ascontiguousarray(
                bsel[DI:DI + DH].reshape(NG, 128, 1))
            cw = inputs['conv_w'][g].astype(np.float32)[perm]      # [DI, K]
            cwd = np.zeros((NDT, 128, K_ * 128), np.float32)
            for j in range(NDT):
                for k in range(K_):
                    cwd[j, np.arange(128), k * 128 + np.arange(128)] = \
                        cw[j * 128:(j + 1) * 128, k]
            m[f'convw_{i}'] = cwd.astype(BF)
            # conv bias with the xs-bias folded in:
            # conv(xs + beta) = conv(xs) + beta * sum_k w   (interior exact;
            # beta = ln_b@in_w is zero for this problem anyway)
            beta = bsel[:DI]
            cb = inputs['conv_b'][g].astype(np.float32)[perm] \
                + beta * cw.sum(axis=1)
            m[f'convb_{i}'] = np.ascontiguousarray(cb.reshape(NDT, 128, 1))
            xp = np.concatenate([inputs['xd_w'][g], inputs['xB_w'][g],
                                 inputs['xC_w'][g]], axis=1)[perm]  # [DI, 64]
            m[f'xproj_{i}'] = np.ascontiguousarray(
                xp.astype(np.float32).reshape(NDT, 128, R_ + 2 * N_)).astype(BF)
            m[f'dtpw_{i}'] = np.ascontiguousarray(
                inputs['dtp_w'][g].astype(np.float32)[:, own]).astype(BF)
            m[f'dtpb_{i}'] = np.ascontiguousarray(
                inputs['dtp_b'][g][own].astype(np.float32).reshape(NG, 128, 1))
            A = -np.exp(inputs['A_log'][g].astype(np.float64)).astype(np.float32)
            m[f'acol_{i}'] = np.ascontiguousarray(
                A[own].reshape(NG, 128, N_))                        # [2,128,16]
            m[f'dvec_{i}'] = np.ascontiguousarray(
                inputs['D'][g][own].astype(np.float32).reshape(NG, 128, 1))
            ow = inputs['out_w'][g].astype(np.float32)[own]         # [256, 256]
            m[f'outw_{i}'] = np.ascontiguousarray(ow.reshape(2, 128, DM)).astype(BF)

        mw = inputs['merge_w'].astype(np.float32)                   # [512, 256]
        m['mergew'] = np.ascontiguousarray(
            mw[d * 256:(d + 1) * 256, :].reshape(2, 128, DM)).astype(BF)
        av = np.full((128, 1), 1.0 if dh == 0 else 0.0, np.float32)
        m['alphav'] = av
        m['ones128'] = np.full((128, 1), 1.0 / DM, np.float32)
        cb2 = np.zeros((128, 2), np.float32)
        cb2[:, 0] = 1e-5
        cb2[:, 1] = 1.0
        m['cbias'] = cb2
        m['ones1'] = np.ones((1, 128), BF)
        m['ident'] = np.eye(128, dtype=np.float32).astype(BF)
        in_maps.append(m)
    return in_maps


# ---------------------------------------------------------------- device

def _build_program():
    from contextlib import ExitStack
    import concourse.bass as bass
    import concourse.tile as tile
    from concourse import bacc, mybir

    f32 = mybir.dt.float32

    # Route Exp and Ln to the combined natural_log_exp table set so the
    # softplus (Exp then Ln) and dA-exp phases share one resident table.
    from concourse import hw_specs as _hw
    _orig_tables = _hw.get_activation_tables
    def _filtered_tables(arch):
        t = dict(_orig_tables(arch))
        for drop in ('exp_and_others', 'exp_and_friends', 'natural_log'):
            if drop in t:
                t[drop] = frozenset()   # keep index alignment, never matched
        return t
    bacc.get_activation_tables = _filtered_tables

    nc = bacc.Bacc("TRN2", target_bir_lowering=False, debug=False,
                   num_devices=8)

    f32r = mybir.dt.float32r

    bf16 = mybir.dt.bfloat16

    def din(name, shape, dt=None):
        return nc.dram_tensor(name, list(shape), dt or f32,
                              kind="ExternalInput").ap()

    W = {'x_in': din('x_in', (NXT, 128, S_), f32r)}
    for i in range(NB):
        W[f'inw_{i}'] = din(f'inw_{i}', (2, 128, 768), bf16)
        W[f'zb_{i}'] = din(f'zb_{i}', (NG, 128, 1))
        W[f'convw_{i}'] = din(f'convw_{i}', (NDT, 128, K_ * 128), bf16)
        W[f'convb_{i}'] = din(f'convb_{i}', (NDT, 128, 1))
        W[f'xproj_{i}'] = din(f'xproj_{i}', (NDT, 128, R_ + 2 * N_), bf16)
        W[f'dtpw_{i}'] = din(f'dtpw_{i}', (R_, DH), bf16)
        W[f'dtpb_{i}'] = din(f'dtpb_{i}', (NG, 128, 1))
        W[f'acol_{i}'] = din(f'acol_{i}', (NG, 128, N_))
        W[f'dvec_{i}'] = din(f'dvec_{i}', (NG, 128, 1))
        W[f'outw_{i}'] = din(f'outw_{i}', (2, 128, DM), bf16)
    W['mergew'] = din('mergew', (2, 128, DM), bf16)
    W['alphav'] = din('alphav', (128, 1))
    W['ones128'] = din('ones128', (128, 1), f32r)
    W['cbias'] = din('cbias', (128, 2))
    W['ones1'] = din('ones1', (1, 128), bf16)
    W['ident'] = din('ident', (128, 128), bf16)

    outp = nc.dram_tensor('outp', [DM, S_], f32, kind="ExternalOutput").ap()

    with tile.TileContext(nc) as tc:
        with ExitStack() as ctx:
            _emit(ctx, nc, tc, bass, mybir, f32, W, outp)
    nc.compile()
    return nc


def _emit(ctx, nc, tc, bass, mybir, f32, W, outp):
    EX = ctx.enter_context
    AF = mybir.ActivationFunctionType
    OPa, OPm = mybir.AluOpType.add, mybir.AluOpType.mult
    f32r = mybir.dt.float32r
    bf16 = mybir.dt.bfloat16
    ts = bass.ts

    # ---- pools
    wpool = EX(tc.tile_pool(name="wconst", bufs=1))
    wblk = EX(tc.tile_pool(name="wblk", bufs=1))
    xio = EX(tc.tile_pool(name="xio", bufs=2))          # xT tiles f32r
    lnp = EX(tc.tile_pool(name="lnp", bufs=1))
    stg = EX(tc.tile_pool(name="stg", bufs=2))          # xn / tmp chunks
    xsp = EX(tc.tile_pool(name="xsp", bufs=2))          # xs halo chunks
    xcp = EX(tc.tile_pool(name="xcp", bufs=2))          # xc chunks bf16
    rkp = EX(tc.tile_pool(name="rkp", bufs=2))          # rank rows chunks
    widep = EX(tc.tile_pool(name="widep", bufs=1))      # per-block wide tiles
    bbp = EX(tc.tile_pool(name="bbp", bufs=3))          # Bb/Cb bcast tiles
    dap = EX(tc.tile_pool(name="dap", bufs=5))          # dA (deep, batches Act)
    scp = EX(tc.tile_pool(name="scp", bufs=3))          # dBx/hC
    hp = EX(tc.tile_pool(name="hp", bufs=3))            # h (shared tag)
    cyp = EX(tc.tile_pool(name="cyp", bufs=1))          # half-boundary carries
    pop = EX(tc.tile_pool(name="pop", bufs=2))

    ps_stage = EX(tc.tile_pool(name="ps_stage", bufs=3, space="PSUM"))
    ps_out = EX(tc.tile_pool(name="ps_out", bufs=1, space="PSUM"))
    ps_acc = EX(tc.tile_pool(name="ps_acc", bufs=1, space="PSUM"))
    dram = EX(tc.tile_pool(name="dram", bufs=2, space="DRAM"))

    # ---- constants
    def cw(name, shape, dt=None):
        t = wpool.tile(list(shape), dt or f32, tag=name)
        nc.sync.dma_start(t[:], W[name][:])
        return t

    ones128 = cw('ones128', (128, 1), f32r)
    cbias = cw('cbias', (128, 2))
    ones1 = cw('ones1', (1, 128), bf16)
    ident = cw('ident', (128, 128), bf16)
    alphav = cw('alphav', (128, 1))
    mergew = [wpool.tile([128, DM], bf16, tag=f"mgw{j}", name=f"mgw{j}")
              for j in range(2)]
    for j in range(2):
        nc.sync.dma_start(mergew[j][:], W['mergew'][j])

    # ---- x input (per half-sequence tiles)
    xT = [[xio.tile([128, HW_], f32r, tag=f"xT{j}{hh}", name=f"xTin{j}{hh}")
           for hh in range(2)] for j in range(NXT)]
    for j in range(NXT):
        for hh in range(2):
            nc.sync.dma_start(xT[j][hh][:],
                              W['x_in'][j, :, hh * HW_:(hh + 1) * HW_])

    replica_groups = [[0, 1], [2, 3], [4, 5], [6, 7]]

    from concourse.tile import add_dep_helper
    first_rb = None

    for blk in range(NB):
        last = blk == NB - 1
        # ---- per-block weights (tags read late in the previous block get
        # bufs=2 so this block's loads are not serialized behind them)
        w = {}
        for nm, cnt, shp in [('inw', 2, (128, 768)), ('zb', NG, (128, 1)),
                             ('convw', NDT, (128, K_ * 128)),
                             ('convb', NDT, (128, 1)),
                             ('xproj', NDT, (128, R_ + 2 * N_)),
                             ('dtpb', NG, (128, 1)), ('acol', NG, (128, N_)),
                             ('dvec', NG, (128, 1)), ('outw', 2, (128, DM))]:
            wdt = bf16 if nm in ('inw', 'xproj', 'outw', 'convw') else f32
            wbufs = 2 if nm in ('acol', 'dvec', 'outw') else 1
            tl = []
            for j in range(cnt):
                t = wblk.tile(list(shp), wdt, tag=f"{nm}{j}", bufs=wbufs)
                wdma = nc.sync.dma_start(t[:], W[f'{nm}_{blk}'][j])
                if blk == 1:
                    tc.chain_iter_dep("rbgate", wdma.ins)
                tl.append(t)
            w[nm] = tl
        w['dtpw'] = wblk.tile([R_, DH], bf16, tag="dtpw", name=f"dtpw{blk}")
        last_wdma = nc.sync.dma_start(w['dtpw'][:], W[f'dtpw_{blk}'][:])

        pending_h1 = None
        if blk == 1:
            # deferred h1 readback of the block0 exchange: the DMAs go out
            # now (ordered after this block's weight loads in the SP queue);
            # the residual adds are emitted later, right before the stage
            # needs the h1 residual, so they don't block the DVE queue.
            bounce_out0, xT_prev = deferred_rb
            tc.chain_iter_dep("rbgate", last_wdma.ins)
            pending_h1 = []
            for m in range(NXT):
                p0 = pop.tile([128, HW_], bf16, tag="peer0", bufs=2,
                              name=f"p0_1_{m}")
                p1 = pop.tile([128, HW_], bf16, tag="peer1", bufs=2,
                              name=f"p1_1_{m}")
                rb = nc.sync.dma_start(
                    p0[:], bounce_out0[1][128 * m:128 * (m + 1), :])
                tc.chain_iter_dep("rbgate", rb.ins)
                rb2 = nc.sync.dma_start(
                    p1[:], bounce_out0[1][DM + 128 * m:DM + 128 * (m + 1), :])
                tc.chain_iter_dep("rbgate", rb2.ins)
                pending_h1.append((p0, p1, m))

        # wide per-block tiles (per half h)
        delw = [[widep.tile([128, HW_], bf16, tag=f"delw{g}{h}",
                            name=f"delw{blk}_{g}{h}") for h in range(2)]
                for g in range(NG)]
        dxw = [[widep.tile([128, HW_], bf16, tag=f"dxw{g}{h}",
                           name=f"dxw{blk}_{g}{h}") for h in range(2)]
               for g in range(NG)]
        xcow = [[widep.tile([128, HW_], bf16, tag=f"xcow{g}{h}",
                            name=f"xcow{blk}_{g}{h}") for h in range(2)]
                for g in range(NG)]
        zw = [[widep.tile([128, HW_], bf16, tag=f"zw{g}{h}",
                          name=f"zw{blk}_{g}{h}") for h in range(2)]
              for g in range(NG)]
        bcBC = [widep.tile([R_, HW_], bf16, tag=f"bcBC{h}",
                           name=f"bcBC{blk}_{h}") for h in range(2)]

        # ============================== stage: chunk loop
        xs_prev = [None] * NDT
        xcw_t = [None] * NCH
        for c in range(NCH):
            cs = ts(c, TCk)
            h = c // 2
            hs = ts(c % 2, TCk)    # slice within the half-wide tiles
            if c == 2 and pending_h1 is not None:
                for p0, p1, m in pending_h1:
                    psum12 = stg.tile([128, HW_], f32, tag="psum12", bufs=2,
                                      name=f"ps12_1_{m}")
                    nc.gpsimd.tensor_add(psum12[:], p0[:], p1[:])
                    nc.gpsimd.tensor_add(xT[m][1][:], psum12[:],
                                         xT_prev[m][1][:])
                pending_h1 = None
            # LN stats for this chunk
            st = ps_stage.tile([128, TCk], f32, tag="ps", name=f"st{c}")
            st2 = ps_stage.tile([128, TCk], f32, tag="ps", name=f"st2{c}")
            sq = [stg.tile([128, TCk], f32r, tag="sq", bufs=2, name=f"sq{j}_{c}")
                  for j in range(NXT)]
            for j in range(NXT):
                nc.gpsimd.tensor_mul(sq[j][:], xT[j][h][:, hs], xT[j][h][:, hs])
            for j in range(NXT):
                nc.tensor.matmul(st[0:1, :], ones128[:], xT[j][h][:, hs],
                                 start=(j == 0), stop=(j == NXT - 1))
            for j in range(NXT):
                nc.tensor.matmul(st2[0:1, :], ones128[:], sq[j][:],
                                 start=(j == 0), stop=(j == NXT - 1))
            sumx = lnp.tile([1, TCk], bf16, tag=f"sumx{c % 2}", name=f"sumx{c}")
            rstd = lnp.tile([1, TCk], bf16, tag=f"rstd{c % 2}", name=f"rstd{c}")
            nc.vector.tensor_copy(sumx[:], st[0:1, :])
            var = lnp.tile([1, TCk], f32, tag="var", bufs=2, name=f"var{c}")
            mm = lnp.tile([1, TCk], f32, tag="mm", bufs=2, name=f"mm{c}")
            nc.vector.tensor_mul(mm[:], sumx[:], sumx[:])
            nc.vector.tensor_sub(var[:], st2[0:1, :], mm[:])
            nc.scalar.activation(var[:], var[:], AF.Ln, bias=cbias[0:1, 0:1])
            nc.scalar.activation(rstd[:], var[:], AF.Exp, scale=-0.5)
            mrep = ps_stage.tile([128, TCk], f32, tag="ps", name=f"mrep{c}")
            rrep = ps_stage.tile([128, TCk], f32, tag="ps", name=f"rrep{c}")
            nc.tensor.matmul(mrep[:], ones1[:], sumx[:],
                             start=True, stop=True)
            nc.tensor.matmul(rrep[:], ones1[:], rstd[:],
                             start=True, stop=True)
            mrepS = stg.tile([128, TCk], f32, tag="mrepS", bufs=1)
            rrepS = stg.tile([128, TCk], f32, tag="rrepS", bufs=1)
            nc.vector.tensor_copy(mrepS[:], mrep[:])
            nc.vector.tensor_copy(rrepS[:], rrep[:])
            xn = []
            for j in range(NXT):
                t = stg.tile([128, TCk], bf16, tag=f"xn{j}")
                tf = stg.tile([128, TCk], f32, tag="xnf", name=f"xnf{c}_{j}")
                nc.gpsimd.tensor_sub(tf[:], xT[j][h][:, hs], mrepS[:])
                nc.gpsimd.tensor_mul(t[:], tf[:], rrepS[:])
                xn.append(t)
            # in-proj
            xc_t = [None] * NDT
            for m in range(6):
                ps = ps_stage.tile([128, TCk], f32, tag="ps", name=f"pi{c}_{m}")
                for k in range(2):
                    nc.tensor.matmul(ps[:], w['inw'][k][:, ts(m, 128)],
                                     xn[k][:], start=(k == 0), stop=(k == 1))
                if m < NDT:
                    xt = xsp.tile([128, TCk + 4], bf16, tag=f"xs{m}")
                    nc.vector.tensor_copy(xt[:, 4:], ps[:])
                    if c == 0:
                        nc.vector.memset(xt[:, 0:4].bitcast(mybir.dt.uint16), 0)
                    else:
                        nc.vector.tensor_copy(
                            xt[:, 1:4], xs_prev[m][:, TCk + 1:TCk + 4])
                    psc = ps_stage.tile([128, TCk], f32, tag="ps",
                                        name=f"psc{c}_{m}")
                    for sh in range(K_):
                        nc.tensor.matmul(
                            psc[:], w['convw'][m][:, ts(3 - sh, 128)],
                            xt[:, 4 - sh:TCk + 4 - sh],
                            start=(sh == 0), stop=(sh == K_ - 1))
                    xc = xcp.tile([128, TCk], bf16, tag=f"xc{m}")
                    nc.scalar.activation(xc[:], psc[:], AF.Silu,
                                         bias=w['convb'][m][:])
                    xs_prev[m] = xt
                    xc_t[m] = xc
                    if m < NG:
                        nc.vector.tensor_copy(xcow[m][h][:, hs], xc[:])
                else:
                    g = m - NDT
                    nc.scalar.activation(zw[g][h][:, hs], ps[:], AF.Silu,
                                         bias=w['zb'][g][:])
            # rank projections: one [64, TCk] psum
            psr = ps_stage.tile([128, TCk], f32, tag="ps", name=f"prr{c}")
            for k in range(NDT):
                nc.tensor.matmul(psr[0:R_ + 2 * N_, :], w['xproj'][k][:],
                                 xc_t[k][:], start=(k == 0),
                                 stop=(k == NDT - 1))
            xcw = rkp.tile([R_, TCk], bf16, tag="xcw")
            nc.vector.tensor_copy(xcw[:], psr[0:R_, :])
            nc.vector.tensor_copy(bcBC[h][:, hs], psr[R_:R_ + R_, :])
            xcw_t[c] = xcw
            # delta = softplus(dtpw @ xcw + dtpb), batched per half so the
            # Exp/Ln table is loaded once after the half's Silu run
            if c % 2 == 1:
                for c2 in range(2):
                    hs2 = ts(c2, TCk)
                    for g in range(NG):
                        psd = ps_stage.tile([128, TCk], f32, tag="ps",
                                            name=f"pd{c}_{c2}_{g}")
                        nc.tensor.matmul(psd[:],
                                         w['dtpw'][:, ts(g, 128)],
                                         xcw_t[2 * h + c2][:],
                                         start=True, stop=True)
                        et = stg.tile([128, TCk], f32, tag="sptmp")
                        nc.scalar.activation(et[:], psd[:], AF.Exp,
                                             bias=w['dtpb'][g][:])
                        nc.scalar.activation(delw[g][h][:, hs2], et[:],
                                             AF.Ln, bias=cbias[:, 1:2])

        # ============================== scan phase, per half
        dbc = dram.tile([R_, S_], bf16, tag="dbc")
        accs = [[None] * NG, [None] * NG]
        carry = {}
        bounce_h1_dmas = []
        if blk == 0:
            partial = [[pop.tile([128, HW_], bf16, tag=f"pm{m}{hh}", bufs=1,
                                 name=f"par{m}{hh}") for hh in range(2)]
                       for m in range(NXT)]
            bounce_in = [dram.tile([DM, HW_], bf16, tag=f"bin{hh}",
                                   name=f"bin{hh}") for hh in range(2)]
            bounce_out = [dram.tile([2 * DM, HW_], bf16, tag=f"bout{hh}",
                                    name=f"bout{hh}") for hh in range(2)]
            xT_next = [[xio.tile([128, HW_], f32r, tag=f"xT{j}{hh}",
                                 name=f"xTn{j}{hh}") for hh in range(2)]
                       for j in range(NXT)]
        else:
            preM = [[pop.tile([128, HW_], bf16, tag=f"pm{m}{hh}", bufs=1,
                              name=f"preM{m}{hh}") for hh in range(2)]
                    for m in range(NXT)]
        for h in range(2):
            hsl = slice(h * HW_, (h + 1) * HW_)
            nc.sync.dma_start(dbc[:, hsl], bcBC[h][:])
            # dx = delta * xc (own), wide
            for g in range(NG):
                nc.vector.tensor_mul(dxw[g][h][:], delw[g][h][:],
                                     xcow[g][h][:])
            for n in range(N_):
                Bb = bbp.tile([128, HW_], bf16, tag="Bb")
                nc.sync.dma_start(
                    Bb[:], dbc[n:n + 1, hsl].broadcast_to((128, HW_)))
                Cb = bbp.tile([128, HW_], bf16, tag="Cb")
                cb_dma = nc.sync.dma_start(
                    Cb[:], dbc[N_ + n:N_ + n + 1, hsl].broadcast_to((128, HW_)))
                if blk == 0 and h == 1 and n == N_ - 1:
                    tc.chain_iter_dep("rbgate", cb_dma.ins)
                for g in range(NG):
                    k = n * NG + g
                    dA = dap.tile([128, HW_], bf16, tag="dA")
                    nc.scalar.activation(dA[:], delw[g][h][:], AF.Exp,
                                         scale=w['acol'][g][:, n:n + 1])
                    dBx = scp.tile([128, HW_], bf16, tag="dBx")
                    beng = nc.gpsimd if n >= 11 else nc.vector
                    beng.tensor_mul(dBx[:], dxw[g][h][:], Bb[:])
                    ht = hp.tile([128, HW_], bf16, tag="h")
                    eng = nc.vector if (k % 32) < _SCAN_DVE else nc.gpsimd
                    eng.tensor_tensor_scan(
                        ht[:], dA[:], dBx[:],
                        0.0 if h == 0 else carry[(g, n)][:],
                        op0=OPm, op1=OPa)
                    if h == 0:
                        cy = cyp.tile([128, 1], bf16, tag=f"cy{g}_{n}")
                        nc.gpsimd.tensor_copy(cy[:], ht[:, HW_ - 1:HW_])
                        carry[(g, n)] = cy
                    hC = scp.tile([128, HW_], bf16, tag="hC")
                    heng = nc.gpsimd if n < 9 else nc.vector
                    heng.tensor_mul(hC[:], ht[:], Cb[:])
                    for c2 in range(2):
                        if n == 0 and g == 0 and c2 == 0:
                            pacc = [[ps_acc.tile([128, TCk], f32,
                                                 tag=f"acc{gg}{cc}",
                                                 name=f"pac{blk}{h}_{gg}{cc}")
                                     for cc in range(2)] for gg in range(NG)]
                        nc.tensor.matmul(pacc[g][c2][:], ident[:],
                                         hC[:, ts(c2, TCk)],
                                         start=(n == 0), stop=(n == N_ - 1))
            for g in range(NG):
                acc = widep.tile([128, HW_], bf16, tag=f"acc{g}{h}",
                                 name=f"acc{blk}_{g}{h}")
                accs[h][g] = acc
                for c2 in range(2):
                    nc.scalar.copy(acc[:, ts(c2, TCk)], pacc[g][c2][:])

            # ---- post + out-proj for this half
            gz = []
            for g in range(NG):
                yg = accs[h][g]
                nc.vector.scalar_tensor_tensor(
                    yg[:], xcow[g][h][:], w['dvec'][g][:], yg[:],
                    OPm, OPa)
                nc.vector.tensor_mul(yg[:], yg[:], zw[g][h][:])
                gz.append(yg)
            for c2 in range(2):
                c = 2 * h + c2
                cs = ts(c, TCk)
                hs = ts(c2, TCk)
                for m in range(NXT):
                    pso = ps_out.tile([128, TCk], f32, tag="pso",
                                      name=f"pso{blk}_{c}_{m}")
                    for g in range(NG):
                        nc.tensor.matmul(pso[:], w['outw'][g][:, ts(m, 128)],
                                         gz[g][:, hs], start=(g == 0),
                                         stop=(g == 1))
                    if blk == 0:
                        nc.scalar.copy(partial[m][h][:, hs], pso[:])
                    else:
                        nc.vector.scalar_tensor_tensor(
                            preM[m][h][:, hs], xT[m][h][:, hs], alphav[:],
                            pso[:], OPm, OPa)
            if blk == 0:
                # issue this half's pair exchange (readback deferred so the
                # DMA queue keeps feeding the other half's scan)
                for m in range(NXT):
                    bi_dma = nc.sync.dma_start(
                        bounce_in[h][128 * m:128 * (m + 1), :],
                        partial[m][h][:])
                    if h == 1:
                        bounce_h1_dmas.append(bi_dma)
                if _USE_COLLECTIVE:
                    nc.gpsimd.collective_compute(
                        "AllGather", mybir.AluOpType.bypass,
                        replica_groups=replica_groups,
                        ins=[bounce_in[h][:].opt()],
                        outs=[bounce_out[h][:].opt()])
                else:
                    nc.sync.dma_start(bounce_out[h][0:DM, :], bounce_in[h][:])
                    nc.sync.dma_start(bounce_out[h][DM:2 * DM, :],
                                      bounce_in[h][:])
            else:
                # merge partial for this half
                for c2 in range(2):
                    c = 2 * h + c2
                    cs = ts(c, TCk)
                    hs = ts(c2, TCk)
                    for m in range(NXT):
                        ps = ps_out.tile([128, TCk], f32, tag="pso",
                                         name=f"mg{c}_{m}")
                        for j in range(2):
                            nc.tensor.matmul(ps[:], mergew[j][:, ts(m, 128)],
                                             preM[j][h][:, hs],
                                             start=(j == 0), stop=(j == 1))
                        op = pop.tile([128, TCk], f32, tag="op", bufs=2)
                        nc.scalar.copy(op[:], ps[:])
                        nc.sync.dma_start(outp[128 * m:128 * (m + 1), cs],
                                          op[:])
        if blk == 0:
            # h0 readback + residual adds, gated (chain dep) behind the h1
            # broadcast feed so its AllGather wait cannot starve the second
            # half's scan.  The h1 readback is deferred into block 1.
            for m in range(NXT):
                p0 = pop.tile([128, HW_], bf16, tag="peer0", bufs=2,
                              name=f"p0_0_{m}")
                p1 = pop.tile([128, HW_], bf16, tag="peer1", bufs=2,
                              name=f"p1_0_{m}")
                rb = nc.sync.dma_start(p0[:],
                                       bounce_out[0][128 * m:128 * (m + 1), :])
                tc.chain_iter_dep("rbgate", rb.ins)
                rb2 = nc.sync.dma_start(
                    p1[:], bounce_out[0][DM + 128 * m:DM + 128 * (m + 1), :])
                tc.chain_iter_dep("rbgate", rb2.ins)
                psum12 = stg.tile([128, HW_], f32, tag="psum12", bufs=2,
                                  name=f"ps12_0_{m}")
                nc.gpsimd.tensor_add(psum12[:], p0[:], p1[:])
                nc.gpsimd.tensor_add(xT_next[m][0][:], psum12[:],
                                     xT[m][0][:])
            deferred_rb = (bounce_out, xT)
            xT = xT_next


# ---------------------------------------------------------------- entry

def kernel(**inputs) -> np.ndarray:
    from concourse.bass_utils import run_bass_kernel_spmd

    if 'nc' not in _BUILD_CACHE:
        _BUILD_CACHE['nc'] = _build_program()
    nc = _BUILD_CACHE['nc']

    in_maps = _host_inputs(inputs)
    res = run_bass_kernel_spmd(nc, in_maps, core_ids=list(range(8)))
    _BUILD_CACHE['last_res'] = res
    parts = [r['outp'] for r in res.results]   # [256, 2048] each

    out = np.zeros((B_, S_, DM), np.float32)
    for b in range(B_):
        acc = np.zeros((DM, S_), np.float32)
        for dh in range(2):
            acc += parts[0 * 4 + b * 2 + dh]               # fwd
            acc += parts[1 * 4 + b * 2 + dh][:, ::-1]      # bwd
        out[b] = acc.T
    return out


# revision 68
# speedup vs baseline: 1.0141x; 1.0055x over previous
"""Trainium2 Bass kernel for nn_BidirectionalMamba.

Self-contained: hardcodes shapes from the problem spec.

Sharding (8 cores): core = dir*4 + batch*2 + dhalf
  - dir   in {0,1}: forward chain (blocks 0,1) / backward chain (blocks 2,3,
            fed time-reversed input, un-reversed on host)
  - batch in {0,1}
  - dhalf in {0,1}: each core owns 256 of 512 d_inner channels for the scan
            path; stage-0 (LN/in-proj/conv/rank projections) is replicated
            within the pair.

Scan layout: per-(group, state) tiles [128 channels, T] with the sequence in
the free dim.  dA_n = exp(A[:,n] * delta) comes straight from the scalar
engine (per-partition scale vector), dBx/hC are wide bf16 multiplies on DVE
against B/C rows broadcast to 128 partitions via DMA (DRAM bounce with a
stride-0 source), and the recurrence itself is tensor_tensor_scan on the
Pool engine.  y accumulates over the 16 states with wide bf16 adds.

Cross-core: only the block0->block1 boundary needs the d-half pair's
out-projection partials; that is ONE AllGather per chain (bf16, two
half-sequence pieces so it pipelines with the scan).  The block1 output is
never exchanged: each core multiplies (alpha*residual + own partial) by the
full per-direction merge weight and the host sums the four per-core merge
partials per batch.
"""

import numpy as np
import ml_dtypes

BF = ml_dtypes.bfloat16

B_, S_, DM, DI, N_, R_, K_ = 2, 2048, 256, 512, 16, 32, 4
DH = DI // 2            # 256 own channels per core
NB = 2                  # blocks per chain (per core)
TCk = 512               # time chunk (psum-sized)
NCH = S_ // TCk         # 4
HW_ = S_ // 2           # half-sequence width for wide ops
NXT = DM // 128         # 2 x-tiles
NDT = DI // 128         # 4 full-d tiles
NG = DH // 128          # 2 own 128-channel groups

_BUILD_CACHE = {}
import os
_USE_COLLECTIVE = os.environ.get("NO_CC", "") == ""
# how many of the 32 (g,n) scans per block run on DVE instead of Pool
_SCAN_DVE = int(os.environ.get("SCAN_DVE", "32"))
# how many of the 16 n's get dA built by DVE squaring instead of Act exp
_POW_DVE = int(os.environ.get("POW_DVE", "0"))


# ---------------------------------------------------------------- host prep

def _host_inputs(inputs):
    x = np.ascontiguousarray(inputs['x'], dtype=np.float32)        # [B,S,DM]
    in_maps = []
    for core in range(8):
        d = core // 4          # dir
        b = (core // 2) % 2    # batch
        dh = core % 2          # d-half
        # channel permutation: own 256 first
        perm = np.concatenate([np.arange(dh * DH, (dh + 1) * DH),
                               np.arange((1 - dh) * DH, (2 - dh) * DH)])
        own = slice(dh * DH, (dh + 1) * DH)
        m = {}
        xb = x[b] if d == 0 else x[b, ::-1]
        m['x_in'] = np.ascontiguousarray(xb.T).reshape(NXT, 128, S_)

        for i in range(NB):
            g = 2 * d + i      # global block index
            ln_g = inputs['ln_g'][g].astype(np.float32)
            ln_b = inputs['ln_b'][g].astype(np.float32)
            in_w = inputs['in_w'][g].astype(np.float32)            # [DM, 2DI]
            w_scaled = ln_g[:, None] * in_w
            bvec = ln_b @ in_w                                     # [2DI]
            # columns: xs (512, permuted own-first) + z own (256)
            cols = np.concatenate([perm, DI + perm[:DH]])
            wsel = w_scaled[:, cols]                               # [256, 768]
            bsel = bvec[cols]                                      # [768]
            m[f'inw_{i}'] = np.ascontiguousarray(wsel.reshape(2, 128, 768)).astype(BF)
            # z bias (own 256)
            m[f'zb_{i}'] = np.ascontiguousarray(
                bsel[DI:DI + DH].reshape(NG, 128, 1))
            cw = inputs['conv_w'][g].astype(np.float32)[perm]      # [DI, K]
            cwd = np.zeros((NDT, 128, K_ * 128), np.float32)
            for j in range(NDT):
                for k in range(K_):
                    cwd[j, np.arange(128), k * 128 + np.arange(128)] = \
                        cw[j * 128:(j + 1) * 128, k]
            m[f'convw_{i}'] = cwd.astype(BF)
            # conv bias with the xs-bias folded in:
            # conv(xs + beta) = conv(xs) + beta * sum_k w   (interior exact;
            # beta = ln_b@in_w is zero for this problem anyway)
            beta = bsel[:DI]
            cb = inputs['conv_b'][g].astype(np.float32)[perm] \
                + beta * cw.sum(axis=1)
            m[f'convb_{i}'] = np.ascontiguousarray(cb.reshape(NDT, 128, 1))
            xp = np.concatenate([inputs['xd_w'][g], inputs['xB_w'][g],
                                 inputs['xC_w'][g]], axis=1)[perm]  # [DI, 64]
            m[f'xproj_{i}'] = np.ascontiguousarray(
                xp.astype(np.float32).reshape(NDT, 128, R_ + 2 * N_)).astype(BF)
            m[f'dtpw_{i}'] = np.ascontiguousarray(
                inputs['dtp_w'][g].astype(np.float32)[:, own]).astype(BF)
            m[f'dtpb_{i}'] = np.ascontiguousarray(
                inputs['dtp_b'][g][own].astype(np.float32).reshape(NG, 128, 1))
            A = -np.exp(inputs['A_log'][g].astype(np.float64)).astype(np.float32)
            m[f'acol_{i}'] = np.ascontiguousarray(
                A[own].reshape(NG, 128, N_))                        # [2,128,16]
            m[f'dvec_{i}'] = np.ascontiguousarray(
                inputs['D'][g][own].astype(np.float32).reshape(NG, 128, 1))
            ow = inputs['out_w'][g].astype(np.float32)[own]         # [256, 256]
            m[f'outw_{i}'] = np.ascontiguousarray(ow.reshape(2, 128, DM)).astype(BF)

        mw = inputs['merge_w'].astype(np.float32)                   # [512, 256]
        m['mergew'] = np.ascontiguousarray(
            mw[d * 256:(d + 1) * 256, :].reshape(2, 128, DM)).astype(BF)
        av = np.full((128, 1), 1.0 if dh == 0 else 0.0, np.float32)
        m['alphav'] = av
        m['ones128'] = np.full((128, 1), 1.0 / DM, np.float32)
        cb2 = np.zeros((128, 2), np.float32)
        cb2[:, 0] = 1e-5
        cb2[:, 1] = 1.0
        m['cbias'] = cb2
        m['ones1'] = np.ones((1, 128), BF)
        m['ident'] = np.eye(128, dtype=np.float32).astype(BF)
        in_maps.append(m)
    return in_maps


# ---------------------------------------------------------------- device

def _build_program():
    from contextlib import ExitStack
    import concourse.bass as bass
    import concourse.tile as tile
    from concourse import bacc, mybir

    f32 = mybir.dt.float32

    # Route Exp and Ln to the combined natural_log_exp table set so the
    # softplus (Exp then Ln) and dA-exp phases share one resident table.
    from concourse import hw_specs as _hw
    _orig_tables = _hw.get_activation_tables
    def _filtered_tables(arch):
        t = dict(_orig_tables(arch))
        for drop in ('exp_and_others', 'exp_and_friends', 'natural_log'):
            if drop in t:
                t[drop] = frozenset()   # keep index alignment, never matched
        return t
    bacc.get_activation_tables = _filtered_tables

    nc = bacc.Bacc("TRN2", target_bir_lowering=False, debug=False,
                   num_devices=8)

    f32r = mybir.dt.float32r

    bf16 = mybir.dt.bfloat16

    def din(name, shape, dt=None):
        return nc.dram_tensor(name, list(shape), dt or f32,
                              kind="ExternalInput").ap()

    W = {'x_in': din('x_in', (NXT, 128, S_), f32r)}
    for i in range(NB):
        W[f'inw_{i}'] = din(f'inw_{i}', (2, 128, 768), bf16)
        W[f'zb_{i}'] = din(f'zb_{i}', (NG, 128, 1))
        W[f'convw_{i}'] = din(f'convw_{i}', (NDT, 128, K_ * 128), bf16)
        W[f'convb_{i}'] = din(f'convb_{i}', (NDT, 128, 1))
        W[f'xproj_{i}'] = din(f'xproj_{i}', (NDT, 128, R_ + 2 * N_), bf16)
        W[f'dtpw_{i}'] = din(f'dtpw_{i}', (R_, DH), bf16)
        W[f'dtpb_{i}'] = din(f'dtpb_{i}', (NG, 128, 1))
        W[f'acol_{i}'] = din(f'acol_{i}', (NG, 128, N_))
        W[f'dvec_{i}'] = din(f'dvec_{i}', (NG, 128, 1))
        W[f'outw_{i}'] = din(f'outw_{i}', (2, 128, DM), bf16)
    W['mergew'] = din('mergew', (2, 128, DM), bf16)
    W['alphav'] = din('alphav', (128, 1))
    W['ones128'] = din('ones128', (128, 1), f32r)
    W['cbias'] = din('cbias', (128, 2))
    W['ones1'] = din('ones1', (1, 128), bf16)
    W['ident'] = din('ident', (128, 128), bf16)

    outp = nc.dram_tensor('outp', [DM, S_], f32, kind="ExternalOutput").ap()

    with tile.TileContext(nc) as tc:
        with ExitStack() as ctx:
            _emit(ctx, nc, tc, bass, mybir, f32, W, outp)
    nc.compile()
    return nc


def _emit(ctx, nc, tc, bass, mybir, f32, W, outp):
    EX = ctx.enter_context
    AF = mybir.ActivationFunctionType
    OPa, OPm = mybir.AluOpType.add, mybir.AluOpType.mult
    f32r = mybir.dt.float32r
    bf16 = mybir.dt.bfloat16
    ts = bass.ts

    # ---- pools
    wpool = EX(tc.tile_pool(name="wconst", bufs=1))
    wblk = EX(tc.tile_pool(name="wblk", bufs=1))
    xio = EX(tc.tile_pool(name="xio", bufs=2))          # xT tiles f32r
    lnp = EX(tc.tile_pool(name="lnp", bufs=1))
    stg = EX(tc.tile_pool(name="stg", bufs=2))          # xn / tmp chunks
    xsp = EX(tc.tile_pool(name="xsp", bufs=2))          # xs halo chunks
    xcp = EX(tc.tile_pool(name="xcp", bufs=2))          # xc chunks bf16
    rkp = EX(tc.tile_pool(name="rkp", bufs=2))          # rank rows chunks
    widep = EX(tc.tile_pool(name="widep", bufs=1))      # per-block wide tiles
    bbp = EX(tc.tile_pool(name="bbp", bufs=3))          # Bb/Cb bcast tiles
    dap = EX(tc.tile_pool(name="dap", bufs=5))          # dA (deep, batches Act)
    scp = EX(tc.tile_pool(name="scp", bufs=3))          # dBx/hC
    hp = EX(tc.tile_pool(name="hp", bufs=3))            # h (shared tag)
    cyp = EX(tc.tile_pool(name="cyp", bufs=1))          # half-boundary carries
    pop = EX(tc.tile_pool(name="pop", bufs=2))

    ps_stage = EX(tc.tile_pool(name="ps_stage", bufs=3, space="PSUM"))
    ps_out = EX(tc.tile_pool(name="ps_out", bufs=1, space="PSUM"))
    ps_acc = EX(tc.tile_pool(name="ps_acc", bufs=1, space="PSUM"))
    dram = EX(tc.tile_pool(name="dram", bufs=2, space="DRAM"))

    # ---- constants
    def cw(name, shape, dt=None):
        t = wpool.tile(list(shape), dt or f32, tag=name)
        nc.sync.dma_start(t[:], W[name][:])
        return t

    ones128 = cw('ones128', (128, 1), f32r)
    cbias = cw('cbias', (128, 2))
    ones1 = cw('ones1', (1, 128), bf16)
    ident = cw('ident', (128, 128), bf16)
    alphav = cw('alphav', (128, 1))
    mergew = [wpool.tile([128, DM], bf16, tag=f"mgw{j}", name=f"mgw{j}")
              for j in range(2)]
    for j in range(2):
        nc.sync.dma_start(mergew[j][:], W['mergew'][j])

    # ---- x input (per half-sequence tiles)
    xT = [[xio.tile([128, HW_], f32r, tag=f"xT{j}{hh}", name=f"xTin{j}{hh}")
           for hh in range(2)] for j in range(NXT)]
    for j in range(NXT):
        for hh in range(2):
            nc.sync.dma_start(xT[j][hh][:],
                              W['x_in'][j, :, hh * HW_:(hh + 1) * HW_])

    replica_groups = [[0, 1], [2, 3], [4, 5], [6, 7]]

    from concourse.tile import add_dep_helper
    first_rb = None

    for blk in range(NB):
        last = blk == NB - 1
        # ---- per-block weights (tags read late in the previous block get
        # bufs=2 so this block's loads are not serialized behind them)
        w = {}
        for nm, cnt, shp in [('inw', 2, (128, 768)), ('zb', NG, (128, 1)),
                             ('convw', NDT, (128, K_ * 128)),
                             ('convb', NDT, (128, 1)),
                             ('xproj', NDT, (128, R_ + 2 * N_)),
                             ('dtpb', NG, (128, 1)), ('acol', NG, (128, N_)),
                             ('dvec', NG, (128, 1)), ('outw', 2, (128, DM))]:
            wdt = bf16 if nm in ('inw', 'xproj', 'outw', 'convw') else f32
            wbufs = 2 if nm in ('acol', 'dvec', 'outw') else 1
            tl = []
            for j in range(cnt):
                t = wblk.tile(list(shp), wdt, tag=f"{nm}{j}", bufs=wbufs)
                wdma = nc.sync.dma_start(t[:], W[f'{nm}_{blk}'][j])
                if blk == 1:
                    tc.chain_iter_dep("rbgate", wdma.ins)
                tl.append(t)
            w[nm] = tl
        w['dtpw'] = wblk.tile([R_, DH], bf16, tag="dtpw", name=f"dtpw{blk}")
        last_wdma = nc.sync.dma_start(w['dtpw'][:], W[f'dtpw_{blk}'][:])

        pending_h1 = None
        if blk == 1:
            # deferred h1 readback of the block0 exchange: the DMAs go out
            # now (ordered after this block's weight loads in the SP queue);
            # the residual adds are emitted later, right before the stage
            # needs the h1 residual, so they don't block the DVE queue.
            bounce_out0, xT_prev = deferred_rb
            tc.chain_iter_dep("rbgate", last_wdma.ins)
            pending_h1 = []
            for m in range(NXT):
                p0 = pop.tile([128, HW_], bf16, tag="peer0", bufs=2,
                              name=f"p0_1_{m}")
                p1 = pop.tile([128, HW_], bf16, tag="peer1", bufs=2,
                              name=f"p1_1_{m}")
                rb = nc.sync.dma_start(
                    p0[:], bounce_out0[1][128 * m:128 * (m + 1), :])
                tc.chain_iter_dep("rbgate", rb.ins)
                rb2 = nc.sync.dma_start(
                    p1[:], bounce_out0[1][DM + 128 * m:DM + 128 * (m + 1), :])
                tc.chain_iter_dep("rbgate", rb2.ins)
                pending_h1.append((p0, p1, m))

        # wide per-block tiles (per half h)
        delw = [[widep.tile([128, HW_], bf16, tag=f"delw{g}{h}",
                            name=f"delw{blk}_{g}{h}") for h in range(2)]
                for g in range(NG)]
        dxw = [[widep.tile([128, HW_], bf16, tag=f"dxw{g}{h}",
                           name=f"dxw{blk}_{g}{h}") for h in range(2)]
               for g in range(NG)]
        xcow = [[widep.tile([128, HW_], bf16, tag=f"xcow{g}{h}",
                            name=f"xcow{blk}_{g}{h}") for h in range(2)]
                for g in range(NG)]
        zw = [[widep.tile([128, HW_], bf16, tag=f"zw{g}{h}",
                          name=f"zw{blk}_{g}{h}") for h in range(2)]
              for g in range(NG)]
        bcBC = [widep.tile([R_, HW_], bf16, tag=f"bcBC{h}",
                           name=f"bcBC{blk}_{h}") for h in range(2)]

        # ============================== stage: chunk loop
        xs_prev = [None] * NDT
        xcw_t = [None] * NCH
        for c in range(NCH):
            cs = ts(c, TCk)
            h = c // 2
            hs = ts(c % 2, TCk)    # slice within the half-wide tiles
            if c == 2 and pending_h1 is not None:
                for p0, p1, m in pending_h1:
                    psum12 = stg.tile([128, HW_], f32, tag="psum12", bufs=2,
                                      name=f"ps12_1_{m}")
                    nc.gpsimd.tensor_add(psum12[:], p0[:], p1[:])
                    nc.gpsimd.tensor_add(xT[m][1][:], psum12[:],
                                         xT_prev[m][1][:])
                pending_h1 = None
            # LN stats for this chunk
            st = ps_stage.tile([128, TCk], f32, tag="ps", name=f"st{c}")
            st2 = ps_stage.tile([128, TCk], f32, tag="ps", name=f"st2{c}")
            sq = [stg.tile([128, TCk], f32r, tag="sq", bufs=2, name=f"sq{j}_{c}")
                  for j in range(NXT)]
            for j in range(NXT):
                nc.gpsimd.tensor_mul(sq[j][:], xT[j][h][:, hs], xT[j][h][:, hs])
            for j in range(NXT):
                nc.tensor.matmul(st[0:1, :], ones128[:], xT[j][h][:, hs],
                                 start=(j == 0), stop=(j == NXT - 1))
            for j in range(NXT):
                nc.tensor.matmul(st2[0:1, :], ones128[:], sq[j][:],
                                 start=(j == 0), stop=(j == NXT - 1))
            sumx = lnp.tile([1, TCk], bf16, tag=f"sumx{c % 2}", name=f"sumx{c}")
            rstd = lnp.tile([1, TCk], bf16, tag=f"rstd{c % 2}", name=f"rstd{c}")
            nc.scalar.copy(sumx[:], st[0:1, :])
            var = lnp.tile([1, TCk], f32, tag="var", bufs=2, name=f"var{c}")
            mm = lnp.tile([1, TCk], f32, tag="mm", bufs=2, name=f"mm{c}")
            nc.vector.tensor_mul(mm[:], sumx[:], sumx[:])
            nc.vector.tensor_sub(var[:], st2[0:1, :], mm[:])
            nc.scalar.activation(var[:], var[:], AF.Ln, bias=cbias[0:1, 0:1])
            nc.scalar.activation(rstd[:], var[:], AF.Exp, scale=-0.5)
            mrep = ps_stage.tile([128, TCk], f32, tag="ps", name=f"mrep{c}")
            rrep = ps_stage.tile([128, TCk], f32, tag="ps", name=f"rrep{c}")
            nc.tensor.matmul(mrep[:], ones1[:], sumx[:],
                             start=True, stop=True)
            nc.tensor.matmul(rrep[:], ones1[:], rstd[:],
                             start=True, stop=True)
            mrepS = stg.tile([128, TCk], f32, tag="mrepS", bufs=1)
            rrepS = stg.tile([128, TCk], f32, tag="rrepS", bufs=1)
            nc.vector.tensor_copy(mrepS[:], mrep[:])
            nc.vector.tensor_copy(rrepS[:], rrep[:])
            xn = []
            for j in range(NXT):
                t = stg.tile([128, TCk], bf16, tag=f"xn{j}")
                tf = stg.tile([128, TCk], f32, tag="xnf", name=f"xnf{c}_{j}")
                nc.gpsimd.tensor_sub(tf[:], xT[j][h][:, hs], mrepS[:])
                nc.gpsimd.tensor_mul(t[:], tf[:], rrepS[:])
                xn.append(t)
            # in-proj
            xc_t = [None] * NDT
            for m in range(6):
                ps = ps_stage.tile([128, TCk], f32, tag="ps", name=f"pi{c}_{m}")
                for k in range(2):
                    nc.tensor.matmul(ps[:], w['inw'][k][:, ts(m, 128)],
                                     xn[k][:], start=(k == 0), stop=(k == 1))
                if m < NDT:
                    xt = xsp.tile([128, TCk + 4], bf16, tag=f"xs{m}")
                    nc.vector.tensor_copy(xt[:, 4:], ps[:])
                    if c == 0:
                        nc.vector.memset(xt[:, 0:4].bitcast(mybir.dt.uint16), 0)
                    else:
                        nc.vector.tensor_copy(
                            xt[:, 1:4], xs_prev[m][:, TCk + 1:TCk + 4])
                    psc = ps_stage.tile([128, TCk], f32, tag="ps",
                                        name=f"psc{c}_{m}")
                    for sh in range(K_):
                        nc.tensor.matmul(
                            psc[:], w['convw'][m][:, ts(3 - sh, 128)],
                            xt[:, 4 - sh:TCk + 4 - sh],
                            start=(sh == 0), stop=(sh == K_ - 1))
                    xc = xcp.tile([128, TCk], bf16, tag=f"xc{m}")
                    nc.scalar.activation(xc[:], psc[:], AF.Silu,
                                         bias=w['convb'][m][:])
                    xs_prev[m] = xt
                    xc_t[m] = xc
                    if m < NG:
                        nc.vector.tensor_copy(xcow[m][h][:, hs], xc[:])
                else:
                    g = m - NDT
                    nc.scalar.activation(zw[g][h][:, hs], ps[:], AF.Silu,
                                         bias=w['zb'][g][:])
            # rank projections: one [64, TCk] psum
            psr = ps_stage.tile([128, TCk], f32, tag="ps", name=f"prr{c}")
            for k in range(NDT):
                nc.tensor.matmul(psr[0:R_ + 2 * N_, :], w['xproj'][k][:],
                                 xc_t[k][:], start=(k == 0),
                                 stop=(k == NDT - 1))
            xcw = rkp.tile([R_, TCk], bf16, tag="xcw")
            nc.vector.tensor_copy(xcw[:], psr[0:R_, :])
            nc.vector.tensor_copy(bcBC[h][:, hs], psr[R_:R_ + R_, :])
            xcw_t[c] = xcw
            # delta = softplus(dtpw @ xcw + dtpb), batched per half so the
            # Exp/Ln table is loaded once after the half's Silu run
            if c % 2 == 1:
                for c2 in range(2):
                    hs2 = ts(c2, TCk)
                    for g in range(NG):
                        psd = ps_stage.tile([128, TCk], f32, tag="ps",
                                            name=f"pd{c}_{c2}_{g}")
                        nc.tensor.matmul(psd[:],
                                         w['dtpw'][:, ts(g, 128)],
                                         xcw_t[2 * h + c2][:],
                                         start=True, stop=True)
                        et = stg.tile([128, TCk], f32, tag="sptmp")
                        nc.scalar.activation(et[:], psd[:], AF.Exp,
                                             bias=w['dtpb'][g][:])
                        nc.scalar.activation(delw[g][h][:, hs2], et[:],
                                             AF.Ln, bias=cbias[:, 1:2])

        # ============================== scan phase, per half
        dbc = dram.tile([R_, S_], bf16, tag="dbc")
        accs = [[None] * NG, [None] * NG]
        carry = {}
        bounce_h1_dmas = []
        if blk == 0:
            partial = [[pop.tile([128, HW_], bf16, tag=f"pm{m}{hh}", bufs=1,
                                 name=f"par{m}{hh}") for hh in range(2)]
                       for m in range(NXT)]
            bounce_in = [dram.tile([DM, HW_], bf16, tag=f"bin{hh}",
                                   name=f"bin{hh}") for hh in range(2)]
            bounce_out = [dram.tile([2 * DM, HW_], bf16, tag=f"bout{hh}",
                                    name=f"bout{hh}") for hh in range(2)]
            xT_next = [[xio.tile([128, HW_], f32r, tag=f"xT{j}{hh}",
                                 name=f"xTn{j}{hh}") for hh in range(2)]
                       for j in range(NXT)]
        else:
            preM = [[pop.tile([128, HW_], bf16, tag=f"pm{m}{hh}", bufs=1,
                              name=f"preM{m}{hh}") for hh in range(2)]
                    for m in range(NXT)]
        for h in range(2):
            hsl = slice(h * HW_, (h + 1) * HW_)
            nc.sync.dma_start(dbc[:, hsl], bcBC[h][:])
            # dx = delta * xc (own), wide
            for g in range(NG):
                nc.vector.tensor_mul(dxw[g][h][:], delw[g][h][:],
                                     xcow[g][h][:])
            for n in range(N_):
                Bb = bbp.tile([128, HW_], bf16, tag="Bb")
                nc.sync.dma_start(
                    Bb[:], dbc[n:n + 1, hsl].broadcast_to((128, HW_)))
                Cb = bbp.tile([128, HW_], bf16, tag="Cb")
                cb_dma = nc.sync.dma_start(
                    Cb[:], dbc[N_ + n:N_ + n + 1, hsl].broadcast_to((128, HW_)))
                if blk == 0 and h == 1 and n == N_ - 1:
                    tc.chain_iter_dep("rbgate", cb_dma.ins)
                for g in range(NG):
                    k = n * NG + g
                    dA = dap.tile([128, HW_], bf16, tag="dA")
                    nc.scalar.activation(dA[:], delw[g][h][:], AF.Exp,
                                         scale=w['acol'][g][:, n:n + 1])
                    dBx = scp.tile([128, HW_], bf16, tag="dBx")
                    beng = nc.gpsimd if n >= 11 else nc.vector
                    beng.tensor_mul(dBx[:], dxw[g][h][:], Bb[:])
                    ht = hp.tile([128, HW_], bf16, tag="h")
                    eng = nc.vector if (k % 32) < _SCAN_DVE else nc.gpsimd
                    eng.tensor_tensor_scan(
                        ht[:], dA[:], dBx[:],
                        0.0 if h == 0 else carry[(g, n)][:],
                        op0=OPm, op1=OPa)
                    if h == 0:
                        cy = cyp.tile([128, 1], bf16, tag=f"cy{g}_{n}")
                        nc.gpsimd.tensor_copy(cy[:], ht[:, HW_ - 1:HW_])
                        carry[(g, n)] = cy
                    hC = scp.tile([128, HW_], bf16, tag="hC")
                    heng = nc.gpsimd if n < 9 else nc.vector
                    heng.tensor_mul(hC[:], ht[:], Cb[:])
                    for c2 in range(2):
                        if n == 0 and g == 0 and c2 == 0:
                            pacc = [[ps_acc.tile([128, TCk], f32,
                                                 tag=f"acc{gg}{cc}",
                                                 name=f"pac{blk}{h}_{gg}{cc}")
                                     for cc in range(2)] for gg in range(NG)]
                        nc.tensor.matmul(pacc[g][c2][:], ident[:],
                                         hC[:, ts(c2, TCk)],
                                         start=(n == 0), stop=(n == N_ - 1))
            for g in range(NG):
                acc = widep.tile([128, HW_], bf16, tag=f"acc{g}{h}",
                                 name=f"acc{blk}_{g}{h}")
                accs[h][g] = acc
                for c2 in range(2):
                    nc.scalar.copy(acc[:, ts(c2, TCk)], pacc[g][c2][:])

            # ---- post + out-proj for this half
            gz = []
            for g in range(NG):
                yg = accs[h][g]
                nc.vector.scalar_tensor_tensor(
                    yg[:], xcow[g][h][:], w['dvec'][g][:], yg[:],
                    OPm, OPa)
                nc.vector.tensor_mul(yg[:], yg[:], zw[g][h][:])
                gz.append(yg)
            for c2 in range(2):
                c = 2 * h + c2
                cs = ts(c, TCk)
                hs = ts(c2, TCk)
                for m in range(NXT):
                    pso = ps_out.tile([128, TCk], f32, tag="pso",
                                      name=f"pso{blk}_{c}_{m}")
                    for g in range(NG):
                        nc.tensor.matmul(pso[:], w['outw'][g][:, ts(m, 128)],
                                         gz[g][:, hs], start=(g == 0),
                                         stop=(g == 1))
                    if blk == 0:
                        nc.scalar.copy(partial[m][h][:, hs], pso[:])
                    else:
                        nc.vector.scalar_tensor_tensor(
                            preM[m][h][:, hs], xT[m][h][:, hs], alphav[:],
                            pso[:], OPm, OPa)
            if blk == 0:
                # issue this half's pair exchange (readback deferred so the
                # DMA queue keeps feeding the other half's scan)
                for m in range(NXT):
                    bi_dma = nc.sync.dma_start(
                        bounce_in[h][128 * m:128 * (m + 1), :],
                        partial[m][h][:])
                    if h == 1:
                        bounce_h1_dmas.append(bi_dma)
                if _USE_COLLECTIVE:
                    nc.gpsimd.collective_compute(
                        "AllGather", mybir.AluOpType.bypass,
                        replica_groups=replica_groups,
                        ins=[bounce_in[h][:].opt()],
                        outs=[bounce_out[h][:].opt()])
                else:
                    nc.sync.dma_start(bounce_out[h][0:DM, :], bounce_in[h][:])
                    nc.sync.dma_start(bounce_out[h][DM:2 * DM, :],
                                      bounce_in[h][:])
            else:
                # merge partial for this half
                for c2 in range(2):
                    c = 2 * h + c2
                    cs = ts(c, TCk)
                    hs = ts(c2, TCk)
                    for m in range(NXT):
                        ps = ps_out.tile([128, TCk], f32, tag="pso",
                                         name=f"mg{c}_{m}")
                        for j in range(2):
                            nc.tensor.matmul(ps[:], mergew[j][:, ts(m, 128)],
                                             preM[j][h][:, hs],
                                             start=(j == 0), stop=(j == 1))
                        op = pop.tile([128, TCk], f32, tag="op", bufs=2)
                        nc.scalar.copy(op[:], ps[:])
                        nc.sync.dma_start(outp[128 * m:128 * (m + 1), cs],
                                          op[:])
        if blk == 0:
            # h0 readback + residual adds, gated (chain dep) behind the h1
            # broadcast feed so its AllGather wait cannot starve the second
            # half's scan.  The h1 readback is deferred into block 1.
            for m in range(NXT):
                p0 = pop.tile([128, HW_], bf16, tag="peer0", bufs=2,
                              name=f"p0_0_{m}")
                p1 = pop.tile([128, HW_], bf16, tag="peer1", bufs=2,
                              name=f"p1_0_{m}")
                rb = nc.sync.dma_start(p0[:],
                                       bounce_out[0][128 * m:128 * (m + 1), :])
                tc.chain_iter_dep("rbgate", rb.ins)
                rb2 = nc.sync.dma_start(
                    p1[:], bounce_out[0][DM + 128 * m:DM + 128 * (m + 1), :])
                tc.chain_iter_dep("rbgate", rb2.ins)
                psum12 = stg.tile([128, HW_], f32, tag="psum12", bufs=2,
                                  name=f"ps12_0_{m}")
                nc.gpsimd.tensor_add(psum12[:], p0[:], p1[:])
                nc.gpsimd.tensor_add(xT_next[m][0][:], psum12[:],
                                     xT[m][0][:])
            deferred_rb = (bounce_out, xT)
            xT = xT_next


# ---------------------------------------------------------------- entry

def kernel(**inputs) -> np.ndarray:
    from concourse.bass_utils import run_bass_kernel_spmd

    if 'nc' not in _BUILD_CACHE:
        _BUILD_CACHE['nc'] = _build_program()
    nc = _BUILD_CACHE['nc']

    in_maps = _host_inputs(inputs)
    res = run_bass_kernel_spmd(nc, in_maps, core_ids=list(range(8)))
    _BUILD_CACHE['last_res'] = res
    parts = [r['outp'] for r in res.results]   # [256, 2048] each

    out = np.zeros((B_, S_, DM), np.float32)
    for b in range(B_):
        acc = np.zeros((DM, S_), np.float32)
        for dh in range(2):
            acc += parts[0 * 4 + b * 2 + dh]               # fwd
            acc += parts[1 * 4 + b * 2 + dh][:, ::-1]      # bwd
        out[b] = acc.T
    return out


# revision 77
# speedup vs baseline: 1.0205x; 1.0063x over previous
"""Trainium2 Bass kernel for nn_BidirectionalMamba.

Self-contained: hardcodes shapes from the problem spec.

Sharding (8 cores): core = dir*4 + batch*2 + dhalf
  - dir   in {0,1}: forward chain (blocks 0,1) / backward chain (blocks 2,3,
            fed time-reversed input, un-reversed on host)
  - batch in {0,1}
  - dhalf in {0,1}: each core owns 256 of 512 d_inner channels for the scan
            path; stage-0 (LN/in-proj/conv/rank projections) is replicated
            within the pair.

Scan layout: per-(group, state) tiles [128 channels, T] with the sequence in
the free dim.  dA_n = exp(A[:,n] * delta) comes straight from the scalar
engine (per-partition scale vector), dBx/hC are wide bf16 multiplies on DVE
against B/C rows broadcast to 128 partitions via DMA (DRAM bounce with a
stride-0 source), and the recurrence itself is tensor_tensor_scan on the
Pool engine.  y accumulates over the 16 states with wide bf16 adds.

Cross-core: only the block0->block1 boundary needs the d-half pair's
out-projection partials; that is ONE AllGather per chain (bf16, two
half-sequence pieces so it pipelines with the scan).  The block1 output is
never exchanged: each core multiplies (alpha*residual + own partial) by the
full per-direction merge weight and the host sums the four per-core merge
partials per batch.
"""

import numpy as np
import ml_dtypes

BF = ml_dtypes.bfloat16

B_, S_, DM, DI, N_, R_, K_ = 2, 2048, 256, 512, 16, 32, 4
DH = DI // 2            # 256 own channels per core
NB = 2                  # blocks per chain (per core)
TCk = 512               # time chunk (psum-sized)
NCH = S_ // TCk         # 4
HW_ = S_ // 2           # half-sequence width for wide ops
NXT = DM // 128         # 2 x-tiles
NDT = DI // 128         # 4 full-d tiles
NG = DH // 128          # 2 own 128-channel groups

_BUILD_CACHE = {}
import os
_USE_COLLECTIVE = os.environ.get("NO_CC", "") == ""
# how many of the 32 (g,n) scans per block run on DVE instead of Pool
_SCAN_DVE = int(os.environ.get("SCAN_DVE", "32"))
# how many of the 16 n's get dA built by DVE squaring instead of Act exp
_POW_DVE = int(os.environ.get("POW_DVE", "0"))


# ---------------------------------------------------------------- host prep

def _host_inputs(inputs):
    x = np.ascontiguousarray(inputs['x'], dtype=np.float32)        # [B,S,DM]
    in_maps = []
    for core in range(8):
        d = core // 4          # dir
        b = (core // 2) % 2    # batch
        dh = core % 2          # d-half
        # channel permutation: own 256 first
        perm = np.concatenate([np.arange(dh * DH, (dh + 1) * DH),
                               np.arange((1 - dh) * DH, (2 - dh) * DH)])
        own = slice(dh * DH, (dh + 1) * DH)
        m = {}
        xb = x[b] if d == 0 else x[b, ::-1]
        m['x_in'] = np.ascontiguousarray(xb.T).reshape(NXT, 128, S_)

        for i in range(NB):
            g = 2 * d + i      # global block index
            ln_g = inputs['ln_g'][g].astype(np.float32)
            ln_b = inputs['ln_b'][g].astype(np.float32)
            in_w = inputs['in_w'][g].astype(np.float32)            # [DM, 2DI]
            w_scaled = ln_g[:, None] * in_w
            bvec = ln_b @ in_w                                     # [2DI]
            # columns: xs (512, permuted own-first) + z own (256)
            cols = np.concatenate([perm, DI + perm[:DH]])
            wsel = w_scaled[:, cols]                               # [256, 768]
            bsel = bvec[cols]                                      # [768]
            m[f'inw_{i}'] = np.ascontiguousarray(wsel.reshape(2, 128, 768)).astype(BF)
            # z bias (own 256)
            m[f'zb_{i}'] = np.ascontiguousarray(
                bsel[DI:DI + DH].reshape(NG, 128, 1))
            cw = inputs['conv_w'][g].astype(np.float32)[perm]      # [DI, K]
            cwd = np.zeros((NDT, 128, K_ * 128), np.float32)
            for j in range(NDT):
                for k in range(K_):
                    cwd[j, np.arange(128), k * 128 + np.arange(128)] = \
                        cw[j * 128:(j + 1) * 128, k]
            m[f'convw_{i}'] = cwd.astype(BF)
            # conv bias with the xs-bias folded in:
            # conv(xs + beta) = conv(xs) + beta * sum_k w   (interior exact;
            # beta = ln_b@in_w is zero for this problem anyway)
            beta = bsel[:DI]
            cb = inputs['conv_b'][g].astype(np.float32)[perm] \
                + beta * cw.sum(axis=1)
            m[f'convb_{i}'] = np.ascontiguousarray(cb.reshape(NDT, 128, 1))
            xp = np.concatenate([inputs['xd_w'][g], inputs['xB_w'][g],
                                 inputs['xC_w'][g]], axis=1)[perm]  # [DI, 64]
            m[f'xproj_{i}'] = np.ascontiguousarray(
                xp.astype(np.float32).reshape(NDT, 128, R_ + 2 * N_)).astype(BF)
            m[f'dtpw_{i}'] = np.ascontiguousarray(
                inputs['dtp_w'][g].astype(np.float32)[:, own]).astype(BF)
            m[f'dtpb_{i}'] = np.ascontiguousarray(
                inputs['dtp_b'][g][own].astype(np.float32).reshape(NG, 128, 1))
            A = -np.exp(inputs['A_log'][g].astype(np.float64)).astype(np.float32)
            m[f'acol_{i}'] = np.ascontiguousarray(
                A[own].reshape(NG, 128, N_))                        # [2,128,16]
            m[f'dvec_{i}'] = np.ascontiguousarray(
                inputs['D'][g][own].astype(np.float32).reshape(NG, 128, 1))
            ow = inputs['out_w'][g].astype(np.float32)[own]         # [256, 256]
            m[f'outw_{i}'] = np.ascontiguousarray(ow.reshape(2, 128, DM)).astype(BF)

        mw = inputs['merge_w'].astype(np.float32)                   # [512, 256]
        m['mergew'] = np.ascontiguousarray(
            mw[d * 256:(d + 1) * 256, :].reshape(2, 128, DM)).astype(BF)
        av = np.full((128, 1), 1.0 if dh == 0 else 0.0, np.float32)
        m['alphav'] = av
        m['ones128'] = np.full((128, 1), 1.0 / DM, np.float32)
        cb2 = np.zeros((128, 2), np.float32)
        cb2[:, 0] = 1e-5
        cb2[:, 1] = 1.0
        m['cbias'] = cb2
        m['ones1'] = np.ones((1, 128), BF)
        m['ident'] = np.eye(128, dtype=np.float32).astype(BF)
        in_maps.append(m)
    return in_maps


# ---------------------------------------------------------------- device

def _build_program():
    from contextlib import ExitStack
    import concourse.bass as bass
    import concourse.tile as tile
    from concourse import bacc, mybir

    f32 = mybir.dt.float32

    # Route Exp and Ln to the combined natural_log_exp table set so the
    # softplus (Exp then Ln) and dA-exp phases share one resident table.
    from concourse import hw_specs as _hw
    _orig_tables = _hw.get_activation_tables
    def _filtered_tables(arch):
        t = dict(_orig_tables(arch))
        for drop in ('exp_and_others', 'exp_and_friends', 'natural_log'):
            if drop in t:
                t[drop] = frozenset()   # keep index alignment, never matched
        return t
    bacc.get_activation_tables = _filtered_tables

    nc = bacc.Bacc("TRN2", target_bir_lowering=False, debug=False,
                   num_devices=8)

    f32r = mybir.dt.float32r

    bf16 = mybir.dt.bfloat16

    def din(name, shape, dt=None):
        return nc.dram_tensor(name, list(shape), dt or f32,
                              kind="ExternalInput").ap()

    W = {'x_in': din('x_in', (NXT, 128, S_), f32r)}
    for i in range(NB):
        W[f'inw_{i}'] = din(f'inw_{i}', (2, 128, 768), bf16)
        W[f'zb_{i}'] = din(f'zb_{i}', (NG, 128, 1))
        W[f'convw_{i}'] = din(f'convw_{i}', (NDT, 128, K_ * 128), bf16)
        W[f'convb_{i}'] = din(f'convb_{i}', (NDT, 128, 1))
        W[f'xproj_{i}'] = din(f'xproj_{i}', (NDT, 128, R_ + 2 * N_), bf16)
        W[f'dtpw_{i}'] = din(f'dtpw_{i}', (R_, DH), bf16)
        W[f'dtpb_{i}'] = din(f'dtpb_{i}', (NG, 128, 1))
        W[f'acol_{i}'] = din(f'acol_{i}', (NG, 128, N_))
        W[f'dvec_{i}'] = din(f'dvec_{i}', (NG, 128, 1))
        W[f'outw_{i}'] = din(f'outw_{i}', (2, 128, DM), bf16)
    W['mergew'] = din('mergew', (2, 128, DM), bf16)
    W['alphav'] = din('alphav', (128, 1))
    W['ones128'] = din('ones128', (128, 1), f32r)
    W['cbias'] = din('cbias', (128, 2))
    W['ones1'] = din('ones1', (1, 128), bf16)
    W['ident'] = din('ident', (128, 128), bf16)

    outp = nc.dram_tensor('outp', [DM, S_], f32, kind="ExternalOutput").ap()

    with tile.TileContext(nc) as tc:
        with ExitStack() as ctx:
            _emit(ctx, nc, tc, bass, mybir, f32, W, outp)
    nc.compile()
    return nc


def _emit(ctx, nc, tc, bass, mybir, f32, W, outp):
    EX = ctx.enter_context
    AF = mybir.ActivationFunctionType
    OPa, OPm = mybir.AluOpType.add, mybir.AluOpType.mult
    f32r = mybir.dt.float32r
    bf16 = mybir.dt.bfloat16
    ts = bass.ts

    # ---- pools
    wpool = EX(tc.tile_pool(name="wconst", bufs=1))
    wblk = EX(tc.tile_pool(name="wblk", bufs=1))
    xio = EX(tc.tile_pool(name="xio", bufs=2))          # xT tiles f32r
    lnp = EX(tc.tile_pool(name="lnp", bufs=1))
    stg = EX(tc.tile_pool(name="stg", bufs=2))          # xn / tmp chunks
    xsp = EX(tc.tile_pool(name="xsp", bufs=2))          # xs halo chunks
    xcp = EX(tc.tile_pool(name="xcp", bufs=2))          # xc chunks bf16
    rkp = EX(tc.tile_pool(name="rkp", bufs=2))          # rank rows chunks
    widep = EX(tc.tile_pool(name="widep", bufs=1))      # per-block wide tiles
    bbp = EX(tc.tile_pool(name="bbp", bufs=3))          # Bb/Cb bcast tiles
    dap = EX(tc.tile_pool(name="dap", bufs=5))          # dA (deep, batches Act)
    scp = EX(tc.tile_pool(name="scp", bufs=4))          # dBx/hC
    hp = EX(tc.tile_pool(name="hp", bufs=3))            # h (shared tag)
    cyp = EX(tc.tile_pool(name="cyp", bufs=1))          # half-boundary carries
    pop = EX(tc.tile_pool(name="pop", bufs=2))

    ps_stage = EX(tc.tile_pool(name="ps_stage", bufs=3, space="PSUM"))
    ps_out = EX(tc.tile_pool(name="ps_out", bufs=1, space="PSUM"))
    ps_acc = EX(tc.tile_pool(name="ps_acc", bufs=1, space="PSUM"))
    dram = EX(tc.tile_pool(name="dram", bufs=2, space="DRAM"))

    # ---- constants
    def cw(name, shape, dt=None):
        t = wpool.tile(list(shape), dt or f32, tag=name)
        nc.sync.dma_start(t[:], W[name][:])
        return t

    # ---- x input first (per half-sequence tiles): gates the first LN stats
    xT = [[xio.tile([128, HW_], f32r, tag=f"xT{j}{hh}", name=f"xTin{j}{hh}")
           for hh in range(2)] for j in range(NXT)]
    for j in range(NXT):
        for hh in range(2):
            nc.sync.dma_start(xT[j][hh][:],
                              W['x_in'][j, :, hh * HW_:(hh + 1) * HW_])

    ones128 = cw('ones128', (128, 1), f32r)
    cbias = cw('cbias', (128, 2))
    ones1 = cw('ones1', (1, 128), bf16)
    ident = cw('ident', (128, 128), bf16)
    alphav = cw('alphav', (128, 1))
    mergew = [wpool.tile([128, DM], bf16, tag=f"mgw{j}", name=f"mgw{j}")
              for j in range(2)]
    for j in range(2):
        nc.sync.dma_start(mergew[j][:], W['mergew'][j])

    replica_groups = [[0, 1], [2, 3], [4, 5], [6, 7]]

    from concourse.tile import add_dep_helper
    first_rb = None

    for blk in range(NB):
        last = blk == NB - 1
        # ---- per-block weights (tags read late in the previous block get
        # bufs=2 so this block's loads are not serialized behind them)
        w = {}
        for nm, cnt, shp in [('inw', 2, (128, 768)), ('zb', NG, (128, 1)),
                             ('convw', NDT, (128, K_ * 128)),
                             ('convb', NDT, (128, 1)),
                             ('xproj', NDT, (128, R_ + 2 * N_)),
                             ('dtpb', NG, (128, 1)), ('acol', NG, (128, N_)),
                             ('dvec', NG, (128, 1)), ('outw', 2, (128, DM))]:
            wdt = bf16 if nm in ('inw', 'xproj', 'outw', 'convw') else f32
            wbufs = 2 if nm in ('acol', 'dvec', 'outw') else 1
            tl = []
            for j in range(cnt):
                t = wblk.tile(list(shp), wdt, tag=f"{nm}{j}", bufs=wbufs)
                wdma = nc.sync.dma_start(t[:], W[f'{nm}_{blk}'][j])
                if blk == 1:
                    tc.chain_iter_dep("rbgate", wdma.ins)
                tl.append(t)
            w[nm] = tl
        w['dtpw'] = wblk.tile([R_, DH], bf16, tag="dtpw", name=f"dtpw{blk}")
        last_wdma = nc.sync.dma_start(w['dtpw'][:], W[f'dtpw_{blk}'][:])

        pending_h1 = None
        if blk == 1:
            # deferred h1 readback of the block0 exchange: the DMAs go out
            # now (ordered after this block's weight loads in the SP queue);
            # the residual adds are emitted later, right before the stage
            # needs the h1 residual, so they don't block the DVE queue.
            bounce_out0, xT_prev = deferred_rb
            tc.chain_iter_dep("rbgate", last_wdma.ins)
            pending_h1 = []
            for m in range(NXT):
                p0 = pop.tile([128, HW_], bf16, tag="peer0", bufs=2,
                              name=f"p0_1_{m}")
                p1 = pop.tile([128, HW_], bf16, tag="peer1", bufs=2,
                              name=f"p1_1_{m}")
                rb = nc.sync.dma_start(
                    p0[:], bounce_out0[1][128 * m:128 * (m + 1), :])
                tc.chain_iter_dep("rbgate", rb.ins)
                rb2 = nc.sync.dma_start(
                    p1[:], bounce_out0[1][DM + 128 * m:DM + 128 * (m + 1), :])
                tc.chain_iter_dep("rbgate", rb2.ins)
                pending_h1.append((p0, p1, m))

        # wide per-block tiles (per half h)
        delw = [[widep.tile([128, HW_], bf16, tag=f"delw{g}{h}",
                            name=f"delw{blk}_{g}{h}") for h in range(2)]
                for g in range(NG)]
        dxw = [[widep.tile([128, HW_], bf16, tag=f"dxw{g}{h}",
                           name=f"dxw{blk}_{g}{h}") for h in range(2)]
               for g in range(NG)]
        xcow = [[widep.tile([128, HW_], bf16, tag=f"xcow{g}{h}",
                            name=f"xcow{blk}_{g}{h}") for h in range(2)]
                for g in range(NG)]
        zw = [[widep.tile([128, HW_], bf16, tag=f"zw{g}{h}",
                          name=f"zw{blk}_{g}{h}") for h in range(2)]
              for g in range(NG)]
        bcBC = [widep.tile([R_, HW_], bf16, tag=f"bcBC{h}",
                           name=f"bcBC{blk}_{h}") for h in range(2)]

        # ============================== stage: chunk loop
        xs_prev = [None] * NDT
        xcw_t = [None] * NCH
        for c in range(NCH):
            cs = ts(c, TCk)
            h = c // 2
            hs = ts(c % 2, TCk)    # slice within the half-wide tiles
            if c == 2 and pending_h1 is not None:
                for p0, p1, m in pending_h1:
                    psum12 = stg.tile([128, HW_], f32, tag="psum12", bufs=2,
                                      name=f"ps12_1_{m}")
                    nc.gpsimd.tensor_add(psum12[:], p0[:], p1[:])
                    nc.gpsimd.tensor_add(xT[m][1][:], psum12[:],
                                         xT_prev[m][1][:])
                pending_h1 = None
            # LN stats for this chunk
            st = ps_stage.tile([128, TCk], f32, tag="ps", name=f"st{c}")
            st2 = ps_stage.tile([128, TCk], f32, tag="ps", name=f"st2{c}")
            sq = [stg.tile([128, TCk], f32r, tag="sq", bufs=2, name=f"sq{j}_{c}")
                  for j in range(NXT)]
            for j in range(NXT):
                nc.gpsimd.tensor_mul(sq[j][:], xT[j][h][:, hs], xT[j][h][:, hs])
            for j in range(NXT):
                nc.tensor.matmul(st[0:1, :], ones128[:], xT[j][h][:, hs],
                                 start=(j == 0), stop=(j == NXT - 1))
            for j in range(NXT):
                nc.tensor.matmul(st2[0:1, :], ones128[:], sq[j][:],
                                 start=(j == 0), stop=(j == NXT - 1))
            sumx = lnp.tile([1, TCk], bf16, tag=f"sumx{c % 2}", name=f"sumx{c}")
            rstd = lnp.tile([1, TCk], bf16, tag=f"rstd{c % 2}", name=f"rstd{c}")
            nc.scalar.copy(sumx[:], st[0:1, :])
            var = lnp.tile([1, TCk], f32, tag="var", bufs=1, name=f"var{c}")
            mm = lnp.tile([1, TCk], f32, tag="mm", bufs=1, name=f"mm{c}")
            nc.vector.tensor_mul(mm[:], sumx[:], sumx[:])
            nc.vector.tensor_sub(var[:], st2[0:1, :], mm[:])
            nc.scalar.activation(var[:], var[:], AF.Ln, bias=cbias[0:1, 0:1])
            nc.scalar.activation(rstd[:], var[:], AF.Exp, scale=-0.5)
            mrep = ps_stage.tile([128, TCk], f32, tag="ps", name=f"mrep{c}")
            rrep = ps_stage.tile([128, TCk], f32, tag="ps", name=f"rrep{c}")
            nc.tensor.matmul(mrep[:], ones1[:], sumx[:],
                             start=True, stop=True)
            nc.tensor.matmul(rrep[:], ones1[:], rstd[:],
                             start=True, stop=True)
            mrepS = stg.tile([128, TCk], f32, tag="mrepS", bufs=1)
            rrepS = stg.tile([128, TCk], f32, tag="rrepS", bufs=1)
            nc.vector.tensor_copy(mrepS[:], mrep[:])
            nc.vector.tensor_copy(rrepS[:], rrep[:])
            xn = []
            for j in range(NXT):
                t = stg.tile([128, TCk], bf16, tag=f"xn{j}")
                tf = stg.tile([128, TCk], f32, tag="xnf", name=f"xnf{c}_{j}")
                nc.gpsimd.tensor_sub(tf[:], xT[j][h][:, hs], mrepS[:])
                nc.gpsimd.tensor_mul(t[:], tf[:], rrepS[:])
                xn.append(t)
            # in-proj
            xc_t = [None] * NDT
            for m in range(6):
                ps = ps_stage.tile([128, TCk], f32, tag="ps", name=f"pi{c}_{m}")
                for k in range(2):
                    nc.tensor.matmul(ps[:], w['inw'][k][:, ts(m, 128)],
                                     xn[k][:], start=(k == 0), stop=(k == 1))
                if m < NDT:
                    xt = xsp.tile([128, TCk + 4], bf16, tag=f"xs{m}")
                    nc.vector.tensor_copy(xt[:, 4:], ps[:])
                    if c == 0:
                        nc.vector.memset(xt[:, 0:4].bitcast(mybir.dt.uint16), 0)
                    else:
                        nc.vector.tensor_copy(
                            xt[:, 1:4], xs_prev[m][:, TCk + 1:TCk + 4])
                    psc = ps_stage.tile([128, TCk], f32, tag="ps",
                                        name=f"psc{c}_{m}")
                    for sh in range(K_):
                        nc.tensor.matmul(
                            psc[:], w['convw'][m][:, ts(3 - sh, 128)],
                            xt[:, 4 - sh:TCk + 4 - sh],
                            start=(sh == 0), stop=(sh == K_ - 1))
                    xc = xcp.tile([128, TCk], bf16, tag=f"xc{m}")
                    nc.scalar.activation(xc[:], psc[:], AF.Silu,
                                         bias=w['convb'][m][:])
                    xs_prev[m] = xt
                    xc_t[m] = xc
                    if m < NG:
                        nc.vector.tensor_copy(xcow[m][h][:, hs], xc[:])
                else:
                    g = m - NDT
                    nc.scalar.activation(zw[g][h][:, hs], ps[:], AF.Silu,
                                         bias=w['zb'][g][:])
            # rank projections: one [64, TCk] psum
            psr = ps_stage.tile([128, TCk], f32, tag="ps", name=f"prr{c}")
            for k in range(NDT):
                nc.tensor.matmul(psr[0:R_ + 2 * N_, :], w['xproj'][k][:],
                                 xc_t[k][:], start=(k == 0),
                                 stop=(k == NDT - 1))
            xcw = rkp.tile([R_, TCk], bf16, tag="xcw")
            nc.vector.tensor_copy(xcw[:], psr[0:R_, :])
            nc.vector.tensor_copy(bcBC[h][:, hs], psr[R_:R_ + R_, :])
            xcw_t[c] = xcw
            # delta = softplus(dtpw @ xcw + dtpb), batched per half so the
            # Exp/Ln table is loaded once after the half's Silu run
            if c % 2 == 1:
                for c2 in range(2):
                    hs2 = ts(c2, TCk)
                    for g in range(NG):
                        psd = ps_stage.tile([128, TCk], f32, tag="ps",
                                            name=f"pd{c}_{c2}_{g}")
                        nc.tensor.matmul(psd[:],
                                         w['dtpw'][:, ts(g, 128)],
                                         xcw_t[2 * h + c2][:],
                                         start=True, stop=True)
                        et = stg.tile([128, TCk], f32, tag="sptmp")
                        nc.scalar.activation(et[:], psd[:], AF.Exp,
                                             bias=w['dtpb'][g][:])
                        nc.scalar.activation(delw[g][h][:, hs2], et[:],
                                             AF.Ln, bias=cbias[:, 1:2])

        # ============================== scan phase, per half
        dbc = dram.tile([R_, S_], bf16, tag="dbc")
        accs = [[None] * NG, [None] * NG]
        carry = {}
        bounce_h1_dmas = []
        if blk == 0:
            partial = [[pop.tile([128, HW_], bf16, tag=f"pm{m}{hh}", bufs=1,
                                 name=f"par{m}{hh}") for hh in range(2)]
                       for m in range(NXT)]
            bounce_in = [dram.tile([DM, HW_], bf16, tag=f"bin{hh}",
                                   name=f"bin{hh}") for hh in range(2)]
            bounce_out = [dram.tile([2 * DM, HW_], bf16, tag=f"bout{hh}",
                                    name=f"bout{hh}") for hh in range(2)]
            xT_next = [[xio.tile([128, HW_], f32r, tag=f"xT{j}{hh}",
                                 name=f"xTn{j}{hh}") for hh in range(2)]
                       for j in range(NXT)]
        else:
            preM = [[pop.tile([128, HW_], bf16, tag=f"pm{m}{hh}", bufs=1,
                              name=f"preM{m}{hh}") for hh in range(2)]
                    for m in range(NXT)]
        for h in range(2):
            hsl = slice(h * HW_, (h + 1) * HW_)
            nc.sync.dma_start(dbc[:, hsl], bcBC[h][:])
            # dx = delta * xc (own), wide
            for g in range(NG):
                nc.vector.tensor_mul(dxw[g][h][:], delw[g][h][:],
                                     xcow[g][h][:])
            for n in range(N_):
                Bb = bbp.tile([128, HW_], bf16, tag="Bb")
                nc.sync.dma_start(
                    Bb[:], dbc[n:n + 1, hsl].broadcast_to((128, HW_)))
                Cb = bbp.tile([128, HW_], bf16, tag="Cb")
                cb_dma = nc.sync.dma_start(
                    Cb[:], dbc[N_ + n:N_ + n + 1, hsl].broadcast_to((128, HW_)))
                if blk == 0 and h == 1 and n == N_ - 1:
                    tc.chain_iter_dep("rbgate", cb_dma.ins)
                for g in range(NG):
                    k = n * NG + g
                    dA = dap.tile([128, HW_], bf16, tag="dA")
                    nc.scalar.activation(dA[:], delw[g][h][:], AF.Exp,
                                         scale=w['acol'][g][:, n:n + 1])
                    dBx = scp.tile([128, HW_], bf16, tag="dBx")
                    beng = nc.gpsimd if n >= 11 else nc.vector
                    beng.tensor_mul(dBx[:], dxw[g][h][:], Bb[:])
                    ht = hp.tile([128, HW_], bf16, tag="h")
                    eng = nc.vector if (k % 32) < _SCAN_DVE else nc.gpsimd
                    eng.tensor_tensor_scan(
                        ht[:], dA[:], dBx[:],
                        0.0 if h == 0 else carry[(g, n)][:],
                        op0=OPm, op1=OPa)
                    if h == 0:
                        cy = cyp.tile([128, 1], bf16, tag=f"cy{g}_{n}")
                        nc.gpsimd.tensor_copy(cy[:], ht[:, HW_ - 1:HW_])
                        carry[(g, n)] = cy
                    hC = scp.tile([128, HW_], bf16, tag="hC")
                    heng = nc.gpsimd if n < 9 else nc.vector
                    heng.tensor_mul(hC[:], ht[:], Cb[:])
                    for c2 in range(2):
                        if n == 0 and g == 0 and c2 == 0:
                            pacc = [[ps_acc.tile([128, TCk], f32,
                                                 tag=f"acc{gg}{cc}",
                                                 name=f"pac{blk}{h}_{gg}{cc}")
                                     for cc in range(2)] for gg in range(NG)]
                        nc.tensor.matmul(pacc[g][c2][:], ident[:],
                                         hC[:, ts(c2, TCk)],
                                         start=(n == 0), stop=(n == N_ - 1))
            for g in range(NG):
                acc = widep.tile([128, HW_], bf16, tag=f"acc{g}{h}",
                                 name=f"acc{blk}_{g}{h}")
                accs[h][g] = acc
                for c2 in range(2):
                    nc.scalar.copy(acc[:, ts(c2, TCk)], pacc[g][c2][:])

            # ---- post + out-proj for this half
            gz = []
            for g in range(NG):
                yg = accs[h][g]
                nc.vector.scalar_tensor_tensor(
                    yg[:], xcow[g][h][:], w['dvec'][g][:], yg[:],
                    OPm, OPa)
                nc.vector.tensor_mul(yg[:], yg[:], zw[g][h][:])
                gz.append(yg)
            for c2 in range(2):
                c = 2 * h + c2
                cs = ts(c, TCk)
                hs = ts(c2, TCk)
                for m in range(NXT):
                    pso = ps_out.tile([128, TCk], f32, tag="pso",
                                      name=f"pso{blk}_{c}_{m}")
                    for g in range(NG):
                        nc.tensor.matmul(pso[:], w['outw'][g][:, ts(m, 128)],
                                         gz[g][:, hs], start=(g == 0),
                                         stop=(g == 1))
                    if blk == 0:
                        nc.scalar.copy(partial[m][h][:, hs], pso[:])
                    else:
                        nc.vector.scalar_tensor_tensor(
                            preM[m][h][:, hs], xT[m][h][:, hs], alphav[:],
                            pso[:], OPm, OPa)
            if blk == 0:
                # issue this half's pair exchange (readback deferred so the
                # DMA queue keeps feeding the other half's scan)
                for m in range(NXT):
                    bi_dma = nc.sync.dma_start(
                        bounce_in[h][128 * m:128 * (m + 1), :],
                        partial[m][h][:])
                    if h == 1:
                        bounce_h1_dmas.append(bi_dma)
                if _USE_COLLECTIVE:
                    nc.gpsimd.collective_compute(
                        "AllGather", mybir.AluOpType.bypass,
                        replica_groups=replica_groups,
                        ins=[bounce_in[h][:].opt()],
                        outs=[bounce_out[h][:].opt()])
                else:
                    nc.sync.dma_start(bounce_out[h][0:DM, :], bounce_in[h][:])
                    nc.sync.dma_start(bounce_out[h][DM:2 * DM, :],
                                      bounce_in[h][:])
            else:
                # merge partial for this half
                for c2 in range(2):
                    c = 2 * h + c2
                    cs = ts(c, TCk)
                    hs = ts(c2, TCk)
                    for m in range(NXT):
                        ps = ps_out.tile([128, TCk], f32, tag="pso",
                                         name=f"mg{c}_{m}")
                        for j in range(2):
                            nc.tensor.matmul(ps[:], mergew[j][:, ts(m, 128)],
                                             preM[j][h][:, hs],
                                             start=(j == 0), stop=(j == 1))
                        op = pop.tile([128, TCk], f32, tag="op", bufs=2)
                        nc.scalar.copy(op[:], ps[:])
                        nc.sync.dma_start(outp[128 * m:128 * (m + 1), cs],
                                          op[:])
        if blk == 0:
            # h0 readback + residual adds, gated (chain dep) behind the h1
            # broadcast feed so its AllGather wait cannot starve the second
            # half's scan.  The h1 readback is deferred into block 1.
            for m in range(NXT):
                p0 = pop.tile([128, HW_], bf16, tag="peer0", bufs=2,
                              name=f"p0_0_{m}")
                p1 = pop.tile([128, HW_], bf16, tag="peer1", bufs=2,
                              name=f"p1_0_{m}")
                rb = nc.sync.dma_start(p0[:],
                                       bounce_out[0][128 * m:128 * (m + 1), :])
                tc.chain_iter_dep("rbgate", rb.ins)
                rb2 = nc.sync.dma_start(
                    p1[:], bounce_out[0][DM + 128 * m:DM + 128 * (m + 1), :])
                tc.chain_iter_dep("rbgate", rb2.ins)
                psum12 = stg.tile([128, HW_], f32, tag="psum12", bufs=2,
                                  name=f"ps12_0_{m}")
                nc.gpsimd.tensor_add(psum12[:], p0[:], p1[:])
                nc.gpsimd.tensor_add(xT_next[m][0][:], psum12[:],
                                     xT[m][0][:])
            deferred_rb = (bounce_out, xT)
            xT = xT_next


# ---------------------------------------------------------------- entry

def kernel(**inputs) -> np.ndarray:
    from concourse.bass_utils import run_bass_kernel_spmd

    if 'nc' not in _BUILD_CACHE:
        _BUILD_CACHE['nc'] = _build_program()
    nc = _BUILD_CACHE['nc']

    in_maps = _host_inputs(inputs)
    res = run_bass_kernel_spmd(nc, in_maps, core_ids=list(range(8)))
    _BUILD_CACHE['last_res'] = res
    parts = [r['outp'] for r in res.results]   # [256, 2048] each

    out = np.zeros((B_, S_, DM), np.float32)
    for b in range(B_):
        acc = np.zeros((DM, S_), np.float32)
        for dh in range(2):
            acc += parts[0 * 4 + b * 2 + dh]               # fwd
            acc += parts[1 * 4 + b * 2 + dh][:, ::-1]      # bwd
        out[b] = acc.T
    return out


# revision 82
# speedup vs baseline: 1.0359x; 1.0151x over previous
"""Trainium2 Bass kernel for nn_BidirectionalMamba.

Self-contained: hardcodes shapes from the problem spec.

Sharding (8 cores): core = dir*4 + batch*2 + dhalf
  - dir   in {0,1}: forward chain (blocks 0,1) / backward chain (blocks 2,3,
            fed time-reversed input, un-reversed on host)
  - batch in {0,1}
  - dhalf in {0,1}: each core owns 256 of 512 d_inner channels for the scan
            path; stage-0 (LN/in-proj/conv/rank projections) is replicated
            within the pair.

Scan layout: per-(group, state) tiles [128 channels, T] with the sequence in
the free dim.  dA_n = exp(A[:,n] * delta) comes straight from the scalar
engine (per-partition scale vector), dBx/hC are wide bf16 multiplies on DVE
against B/C rows broadcast to 128 partitions via DMA (DRAM bounce with a
stride-0 source), and the recurrence itself is tensor_tensor_scan on the
Pool engine.  y accumulates over the 16 states with wide bf16 adds.

Cross-core: only the block0->block1 boundary needs the d-half pair's
out-projection partials; that is ONE AllGather per chain (bf16, two
half-sequence pieces so it pipelines with the scan).  The block1 output is
never exchanged: each core multiplies (alpha*residual + own partial) by the
full per-direction merge weight and the host sums the four per-core merge
partials per batch.
"""

import numpy as np
import ml_dtypes

BF = ml_dtypes.bfloat16

B_, S_, DM, DI, N_, R_, K_ = 2, 2048, 256, 512, 16, 32, 4
DH = DI // 2            # 256 own channels per core
NB = 2                  # blocks per chain (per core)
TCk = 512               # time chunk (psum-sized)
NCH = S_ // TCk         # 4
HW_ = S_ // 2           # half-sequence width for wide ops
NXT = DM // 128         # 2 x-tiles
NDT = DI // 128         # 4 full-d tiles
NG = DH // 128          # 2 own 128-channel groups

_BUILD_CACHE = {}
import os
_USE_COLLECTIVE = os.environ.get("NO_CC", "") == ""
# how many of the 32 (g,n) scans per block run on DVE instead of Pool
_SCAN_DVE = int(os.environ.get("SCAN_DVE", "32"))
# how many of the 16 n's get dA built by DVE squaring instead of Act exp
_POW_DVE = int(os.environ.get("POW_DVE", "0"))


# ---------------------------------------------------------------- host prep

def _host_inputs(inputs):
    x = np.ascontiguousarray(inputs['x'], dtype=np.float32)        # [B,S,DM]
    in_maps = []
    for core in range(8):
        d = core // 4          # dir
        b = (core // 2) % 2    # batch
        dh = core % 2          # d-half
        # channel permutation: own 256 first
        perm = np.concatenate([np.arange(dh * DH, (dh + 1) * DH),
                               np.arange((1 - dh) * DH, (2 - dh) * DH)])
        own = slice(dh * DH, (dh + 1) * DH)
        m = {}
        xb = x[b] if d == 0 else x[b, ::-1]
        m['x_in'] = np.ascontiguousarray(xb.T).reshape(NXT, 128, S_)

        for i in range(NB):
            g = 2 * d + i      # global block index
            ln_g = inputs['ln_g'][g].astype(np.float32)
            ln_b = inputs['ln_b'][g].astype(np.float32)
            in_w = inputs['in_w'][g].astype(np.float32)            # [DM, 2DI]
            w_scaled = ln_g[:, None] * in_w
            bvec = ln_b @ in_w                                     # [2DI]
            # columns: xs (512, permuted own-first) + z own (256)
            cols = np.concatenate([perm, DI + perm[:DH]])
            wsel = w_scaled[:, cols]                               # [256, 768]
            bsel = bvec[cols]                                      # [768]
            m[f'inw_{i}'] = np.ascontiguousarray(wsel.reshape(2, 128, 768)).astype(BF)
            # z bias (own 256)
            m[f'zb_{i}'] = np.ascontiguousarray(
                bsel[DI:DI + DH].reshape(NG, 128, 1))
            cw = inputs['conv_w'][g].astype(np.float32)[perm]      # [DI, K]
            cwd = np.zeros((NDT, 128, K_ * 128), np.float32)
            for j in range(NDT):
                for k in range(K_):
                    cwd[j, np.arange(128), k * 128 + np.arange(128)] = \
                        cw[j * 128:(j + 1) * 128, k]
            m[f'convw_{i}'] = cwd.astype(BF)
            # conv bias with the xs-bias folded in:
            # conv(xs + beta) = conv(xs) + beta * sum_k w   (interior exact;
            # beta = ln_b@in_w is zero for this problem anyway)
            beta = bsel[:DI]
            cb = inputs['conv_b'][g].astype(np.float32)[perm] \
                + beta * cw.sum(axis=1)
            m[f'convb_{i}'] = np.ascontiguousarray(cb.reshape(NDT, 128, 1))
            xp = np.concatenate([inputs['xd_w'][g], inputs['xB_w'][g],
                                 inputs['xC_w'][g]], axis=1)[perm]  # [DI, 64]
            m[f'xproj_{i}'] = np.ascontiguousarray(
                xp.astype(np.float32).reshape(NDT, 128, R_ + 2 * N_)).astype(BF)
            m[f'dtpw_{i}'] = np.ascontiguousarray(
                inputs['dtp_w'][g].astype(np.float32)[:, own]).astype(BF)
            m[f'dtpb_{i}'] = np.ascontiguousarray(
                inputs['dtp_b'][g][own].astype(np.float32).reshape(NG, 128, 1))
            A = -np.exp(inputs['A_log'][g].astype(np.float64)).astype(np.float32)
            m[f'acol_{i}'] = np.ascontiguousarray(
                A[own].reshape(NG, 128, N_))                        # [2,128,16]
            m[f'dvec_{i}'] = np.ascontiguousarray(
                inputs['D'][g][own].astype(np.float32).reshape(NG, 128, 1))
            ow = inputs['out_w'][g].astype(np.float32)[own]         # [256, 256]
            m[f'outw_{i}'] = np.ascontiguousarray(ow.reshape(2, 128, DM)).astype(BF)

        mw = inputs['merge_w'].astype(np.float32)                   # [512, 256]
        m['mergew'] = np.ascontiguousarray(
            mw[d * 256:(d + 1) * 256, :].reshape(2, 128, DM)).astype(BF)
        av = np.full((128, 1), 1.0 if dh == 0 else 0.0, np.float32)
        m['alphav'] = av
        m['ones128'] = np.full((128, 1), 1.0 / DM, np.float32)
        cb2 = np.zeros((128, 2), np.float32)
        cb2[:, 0] = 1e-5
        cb2[:, 1] = 1.0
        m['cbias'] = cb2
        m['ones1'] = np.ones((1, 128), BF)
        m['ident'] = np.eye(128, dtype=np.float32).astype(BF)
        m['identf'] = np.eye(128, dtype=np.float32)
        in_maps.append(m)
    return in_maps


# ---------------------------------------------------------------- device

def _build_program():
    from contextlib import ExitStack
    import concourse.bass as bass
    import concourse.tile as tile
    from concourse import bacc, mybir

    f32 = mybir.dt.float32

    # Route Exp and Ln to the combined natural_log_exp table set so the
    # softplus (Exp then Ln) and dA-exp phases share one resident table.
    from concourse import hw_specs as _hw
    _orig_tables = _hw.get_activation_tables
    def _filtered_tables(arch):
        t = dict(_orig_tables(arch))
        for drop in ('exp_and_others', 'exp_and_friends', 'natural_log'):
            if drop in t:
                t[drop] = frozenset()   # keep index alignment, never matched
        return t
    bacc.get_activation_tables = _filtered_tables

    nc = bacc.Bacc("TRN2", target_bir_lowering=False, debug=False,
                   num_devices=8)

    f32r = mybir.dt.float32r

    bf16 = mybir.dt.bfloat16

    def din(name, shape, dt=None):
        return nc.dram_tensor(name, list(shape), dt or f32,
                              kind="ExternalInput").ap()

    W = {'x_in': din('x_in', (NXT, 128, S_), f32r)}
    for i in range(NB):
        W[f'inw_{i}'] = din(f'inw_{i}', (2, 128, 768), bf16)
        W[f'zb_{i}'] = din(f'zb_{i}', (NG, 128, 1))
        W[f'convw_{i}'] = din(f'convw_{i}', (NDT, 128, K_ * 128), bf16)
        W[f'convb_{i}'] = din(f'convb_{i}', (NDT, 128, 1))
        W[f'xproj_{i}'] = din(f'xproj_{i}', (NDT, 128, R_ + 2 * N_), bf16)
        W[f'dtpw_{i}'] = din(f'dtpw_{i}', (R_, DH), bf16)
        W[f'dtpb_{i}'] = din(f'dtpb_{i}', (NG, 128, 1))
        W[f'acol_{i}'] = din(f'acol_{i}', (NG, 128, N_))
        W[f'dvec_{i}'] = din(f'dvec_{i}', (NG, 128, 1))
        W[f'outw_{i}'] = din(f'outw_{i}', (2, 128, DM), bf16)
    W['mergew'] = din('mergew', (2, 128, DM), bf16)
    W['alphav'] = din('alphav', (128, 1))
    W['ones128'] = din('ones128', (128, 1), f32r)
    W['cbias'] = din('cbias', (128, 2))
    W['ones1'] = din('ones1', (1, 128), bf16)
    W['ident'] = din('ident', (128, 128), bf16)
    W['identf'] = din('identf', (128, 128), f32r)

    outp = nc.dram_tensor('outp', [DM, S_], f32, kind="ExternalOutput").ap()

    with tile.TileContext(nc) as tc:
        with ExitStack() as ctx:
            _emit(ctx, nc, tc, bass, mybir, f32, W, outp)
    nc.compile()
    return nc


def _emit(ctx, nc, tc, bass, mybir, f32, W, outp):
    EX = ctx.enter_context
    AF = mybir.ActivationFunctionType
    OPa, OPm = mybir.AluOpType.add, mybir.AluOpType.mult
    f32r = mybir.dt.float32r
    bf16 = mybir.dt.bfloat16
    ts = bass.ts

    # ---- pools
    wpool = EX(tc.tile_pool(name="wconst", bufs=1))
    wblk = EX(tc.tile_pool(name="wblk", bufs=1))
    xio = EX(tc.tile_pool(name="xio", bufs=2))          # xT tiles f32r
    lnp = EX(tc.tile_pool(name="lnp", bufs=1))
    stg = EX(tc.tile_pool(name="stg", bufs=2))          # xn / tmp chunks
    xsp = EX(tc.tile_pool(name="xsp", bufs=2))          # xs halo chunks
    xcp = EX(tc.tile_pool(name="xcp", bufs=2))          # xc chunks bf16
    rkp = EX(tc.tile_pool(name="rkp", bufs=2))          # rank rows chunks
    widep = EX(tc.tile_pool(name="widep", bufs=1))      # per-block wide tiles
    bbp = EX(tc.tile_pool(name="bbp", bufs=3))          # Bb/Cb bcast tiles
    dap = EX(tc.tile_pool(name="dap", bufs=5))          # dA (deep, batches Act)
    scp = EX(tc.tile_pool(name="scp", bufs=4))          # dBx/hC
    hp = EX(tc.tile_pool(name="hp", bufs=3))            # h (shared tag)
    cyp = EX(tc.tile_pool(name="cyp", bufs=1))          # half-boundary carries
    pop = EX(tc.tile_pool(name="pop", bufs=2))

    ps_stage = EX(tc.tile_pool(name="ps_stage", bufs=3, space="PSUM"))
    ps_out = EX(tc.tile_pool(name="ps_out", bufs=1, space="PSUM"))
    ps_acc = EX(tc.tile_pool(name="ps_acc", bufs=1, space="PSUM"))
    dram = EX(tc.tile_pool(name="dram", bufs=2, space="DRAM"))

    # ---- constants
    def cw(name, shape, dt=None):
        t = wpool.tile(list(shape), dt or f32, tag=name)
        nc.sync.dma_start(t[:], W[name][:])
        return t

    # ---- x input first (per half-sequence tiles): gates the first LN stats
    xT = [[xio.tile([128, HW_], f32r, tag=f"xT{j}{hh}", name=f"xTin{j}{hh}")
           for hh in range(2)] for j in range(NXT)]
    for j in range(NXT):
        for hh in range(2):
            nc.sync.dma_start(xT[j][hh][:],
                              W['x_in'][j, :, hh * HW_:(hh + 1) * HW_])

    ones128 = cw('ones128', (128, 1), f32r)
    cbias = cw('cbias', (128, 2))
    ones1 = cw('ones1', (1, 128), bf16)
    ident = cw('ident', (128, 128), bf16)
    identf = cw('identf', (128, 128), f32r)
    alphav = cw('alphav', (128, 1))
    mergew = [wpool.tile([128, DM], bf16, tag=f"mgw{j}", name=f"mgw{j}")
              for j in range(2)]
    for j in range(2):
        nc.sync.dma_start(mergew[j][:], W['mergew'][j])

    replica_groups = [[0, 1], [2, 3], [4, 5], [6, 7]]

    from concourse.tile import add_dep_helper
    first_rb = None

    for blk in range(NB):
        last = blk == NB - 1
        # ---- per-block weights (tags read late in the previous block get
        # bufs=2 so this block's loads are not serialized behind them)
        w = {}
        for nm, cnt, shp in [('inw', 2, (128, 768)), ('zb', NG, (128, 1)),
                             ('convw', NDT, (128, K_ * 128)),
                             ('convb', NDT, (128, 1)),
                             ('xproj', NDT, (128, R_ + 2 * N_)),
                             ('dtpb', NG, (128, 1)), ('acol', NG, (128, N_)),
                             ('dvec', NG, (128, 1)), ('outw', 2, (128, DM))]:
            wdt = bf16 if nm in ('inw', 'xproj', 'outw', 'convw') else f32
            wbufs = 2 if nm in ('acol', 'dvec', 'outw') else 1
            tl = []
            for j in range(cnt):
                t = wblk.tile(list(shp), wdt, tag=f"{nm}{j}", bufs=wbufs)
                wdma = nc.sync.dma_start(t[:], W[f'{nm}_{blk}'][j])
                if blk == 1:
                    tc.chain_iter_dep("rbgate", wdma.ins)
                tl.append(t)
            w[nm] = tl
        w['dtpw'] = wblk.tile([R_, DH], bf16, tag="dtpw", name=f"dtpw{blk}")
        last_wdma = nc.sync.dma_start(w['dtpw'][:], W[f'dtpw_{blk}'][:])

        pending_h1 = None
        if blk == 1:
            # deferred h1 readback of the block0 exchange: the DMAs go out
            # now (ordered after this block's weight loads in the SP queue);
            # the residual adds are emitted later, right before the stage
            # needs the h1 residual, so they don't block the DVE queue.
            bounce_out0, xT_prev = deferred_rb
            tc.chain_iter_dep("rbgate", last_wdma.ins)
            pending_h1 = []
            for m in range(NXT):
                p0 = pop.tile([128, HW_], bf16, tag="peer0", bufs=2,
                              name=f"p0_1_{m}")
                p1 = pop.tile([128, HW_], bf16, tag="peer1", bufs=2,
                              name=f"p1_1_{m}")
                rb = nc.sync.dma_start(
                    p0[:], bounce_out0[1][128 * m:128 * (m + 1), :])
                tc.chain_iter_dep("rbgate", rb.ins)
                rb2 = nc.sync.dma_start(
                    p1[:], bounce_out0[1][DM + 128 * m:DM + 128 * (m + 1), :])
                tc.chain_iter_dep("rbgate", rb2.ins)
                pending_h1.append((p0, p1, m))

        # wide per-block tiles (per half h)
        delw = [[widep.tile([128, HW_], bf16, tag=f"delw{g}{h}",
                            name=f"delw{blk}_{g}{h}") for h in range(2)]
                for g in range(NG)]
        dxw = [[widep.tile([128, HW_], bf16, tag=f"dxw{g}{h}",
                           name=f"dxw{blk}_{g}{h}") for h in range(2)]
               for g in range(NG)]
        xcow = [[widep.tile([128, HW_], bf16, tag=f"xcow{g}{h}",
                            name=f"xcow{blk}_{g}{h}") for h in range(2)]
                for g in range(NG)]
        zw = [[widep.tile([128, HW_], bf16, tag=f"zw{g}{h}",
                          name=f"zw{blk}_{g}{h}") for h in range(2)]
              for g in range(NG)]
        bcBC = [widep.tile([R_, HW_], bf16, tag=f"bcBC{h}",
                           name=f"bcBC{blk}_{h}") for h in range(2)]

        # ============================== stage: chunk loop
        xs_prev = [None] * NDT
        xcw_t = [None] * NCH
        for c in range(NCH):
            cs = ts(c, TCk)
            h = c // 2
            hs = ts(c % 2, TCk)    # slice within the half-wide tiles
            if c == 2 and pending_h1 is not None:
                for p0, p1, m in pending_h1:
                    for c2 in range(2):
                        psx = ps_out.tile([128, TCk], f32, tag="pso",
                                          name=f"psx1_{m}_{c2}")
                        nc.tensor.matmul(psx[:], ident[:], p0[:, ts(c2, TCk)],
                                         start=True, stop=False)
                        nc.tensor.matmul(psx[:], ident[:], p1[:, ts(c2, TCk)],
                                         start=False, stop=False)
                        nc.tensor.matmul(psx[:], identf[:],
                                         xT_prev[m][1][:, ts(c2, TCk)],
                                         start=False, stop=True)
                        nc.scalar.copy(xT[m][1][:, ts(c2, TCk)], psx[:])
                pending_h1 = None
            # LN stats for this chunk
            st = ps_stage.tile([128, TCk], f32, tag="ps", name=f"st{c}")
            st2 = ps_stage.tile([128, TCk], f32, tag="ps", name=f"st2{c}")
            sq = [stg.tile([128, TCk], f32r, tag="sq", bufs=2, name=f"sq{j}_{c}")
                  for j in range(NXT)]
            for j in range(NXT):
                nc.gpsimd.tensor_mul(sq[j][:], xT[j][h][:, hs], xT[j][h][:, hs])
            for j in range(NXT):
                nc.tensor.matmul(st[0:1, :], ones128[:], xT[j][h][:, hs],
                                 start=(j == 0), stop=(j == NXT - 1))
            for j in range(NXT):
                nc.tensor.matmul(st2[0:1, :], ones128[:], sq[j][:],
                                 start=(j == 0), stop=(j == NXT - 1))
            sumx = lnp.tile([1, TCk], bf16, tag=f"sumx{c % 2}", name=f"sumx{c}")
            rstd = lnp.tile([1, TCk], bf16, tag=f"rstd{c % 2}", name=f"rstd{c}")
            nc.scalar.copy(sumx[:], st[0:1, :])
            var = lnp.tile([1, TCk], f32, tag="var", bufs=1, name=f"var{c}")
            mm = lnp.tile([1, TCk], f32, tag="mm", bufs=1, name=f"mm{c}")
            nc.vector.tensor_mul(mm[:], sumx[:], sumx[:])
            nc.vector.tensor_sub(var[:], st2[0:1, :], mm[:])
            nc.scalar.activation(var[:], var[:], AF.Ln, bias=cbias[0:1, 0:1])
            nc.scalar.activation(rstd[:], var[:], AF.Exp, scale=-0.5)
            mrep = ps_stage.tile([128, TCk], f32, tag="ps", name=f"mrep{c}")
            rrep = ps_stage.tile([128, TCk], f32, tag="ps", name=f"rrep{c}")
            nc.tensor.matmul(mrep[:], ones1[:], sumx[:],
                             start=True, stop=True)
            nc.tensor.matmul(rrep[:], ones1[:], rstd[:],
                             start=True, stop=True)
            mrepS = stg.tile([128, TCk], f32, tag="mrepS", bufs=1)
            rrepS = stg.tile([128, TCk], f32, tag="rrepS", bufs=1)
            nc.vector.tensor_copy(mrepS[:], mrep[:])
            nc.vector.tensor_copy(rrepS[:], rrep[:])
            xn = []
            for j in range(NXT):
                t = stg.tile([128, TCk], bf16, tag=f"xn{j}")
                tf = stg.tile([128, TCk], f32, tag="xnf", name=f"xnf{c}_{j}")
                nc.gpsimd.tensor_sub(tf[:], xT[j][h][:, hs], mrepS[:])
                nc.gpsimd.tensor_mul(t[:], tf[:], rrepS[:])
                xn.append(t)
            # in-proj
            xc_t = [None] * NDT
            for m in range(6):
                ps = ps_stage.tile([128, TCk], f32, tag="ps", name=f"pi{c}_{m}")
                for k in range(2):
                    nc.tensor.matmul(ps[:], w['inw'][k][:, ts(m, 128)],
                                     xn[k][:], start=(k == 0), stop=(k == 1))
                if m < NDT:
                    xt = xsp.tile([128, TCk + 4], bf16, tag=f"xs{m}")
                    nc.vector.tensor_copy(xt[:, 4:], ps[:])
                    if c == 0:
                        nc.vector.memset(xt[:, 0:4].bitcast(mybir.dt.uint16), 0)
                    else:
                        nc.vector.tensor_copy(
                            xt[:, 1:4], xs_prev[m][:, TCk + 1:TCk + 4])
                    psc = ps_stage.tile([128, TCk], f32, tag="ps",
                                        name=f"psc{c}_{m}")
                    for sh in range(K_):
                        nc.tensor.matmul(
                            psc[:], w['convw'][m][:, ts(3 - sh, 128)],
                            xt[:, 4 - sh:TCk + 4 - sh],
                            start=(sh == 0), stop=(sh == K_ - 1))
                    xc = xcp.tile([128, TCk], bf16, tag=f"xc{m}")
                    nc.scalar.activation(xc[:], psc[:], AF.Silu,
                                         bias=w['convb'][m][:])
                    xs_prev[m] = xt
                    xc_t[m] = xc
                    if m < NG:
                        nc.vector.tensor_copy(xcow[m][h][:, hs], xc[:])
                else:
                    g = m - NDT
                    nc.scalar.activation(zw[g][h][:, hs], ps[:], AF.Silu,
                                         bias=w['zb'][g][:])
            # rank projections: one [64, TCk] psum
            psr = ps_stage.tile([128, TCk], f32, tag="ps", name=f"prr{c}")
            for k in range(NDT):
                nc.tensor.matmul(psr[0:R_ + 2 * N_, :], w['xproj'][k][:],
                                 xc_t[k][:], start=(k == 0),
                                 stop=(k == NDT - 1))
            xcw = rkp.tile([R_, TCk], bf16, tag="xcw")
            nc.vector.tensor_copy(xcw[:], psr[0:R_, :])
            nc.vector.tensor_copy(bcBC[h][:, hs], psr[R_:R_ + R_, :])
            xcw_t[c] = xcw
            # delta = softplus(dtpw @ xcw + dtpb), batched per half so the
            # Exp/Ln table is loaded once after the half's Silu run
            if c % 2 == 1:
                for c2 in range(2):
                    hs2 = ts(c2, TCk)
                    for g in range(NG):
                        psd = ps_stage.tile([128, TCk], f32, tag="ps",
                                            name=f"pd{c}_{c2}_{g}")
                        nc.tensor.matmul(psd[:],
                                         w['dtpw'][:, ts(g, 128)],
                                         xcw_t[2 * h + c2][:],
                                         start=True, stop=True)
                        et = stg.tile([128, TCk], f32, tag="sptmp")
                        nc.scalar.activation(et[:], psd[:], AF.Exp,
                                             bias=w['dtpb'][g][:])
                        nc.scalar.activation(delw[g][h][:, hs2], et[:],
                                             AF.Ln, bias=cbias[:, 1:2])

        # ============================== scan phase, per half
        dbc = dram.tile([R_, S_], bf16, tag="dbc")
        accs = [[None] * NG, [None] * NG]
        carry = {}
        bounce_h1_dmas = []
        if blk == 0:
            partial = [[pop.tile([128, HW_], bf16, tag=f"pm{m}{hh}", bufs=1,
                                 name=f"par{m}{hh}") for hh in range(2)]
                       for m in range(NXT)]
            bounce_in = [dram.tile([DM, HW_], bf16, tag=f"bin{hh}",
                                   name=f"bin{hh}") for hh in range(2)]
            bounce_out = [dram.tile([2 * DM, HW_], bf16, tag=f"bout{hh}",
                                    name=f"bout{hh}") for hh in range(2)]
            xT_next = [[xio.tile([128, HW_], f32r, tag=f"xT{j}{hh}",
                                 name=f"xTn{j}{hh}") for hh in range(2)]
                       for j in range(NXT)]
        else:
            preM = [[pop.tile([128, HW_], bf16, tag=f"pm{m}{hh}", bufs=1,
                              name=f"preM{m}{hh}") for hh in range(2)]
                    for m in range(NXT)]
        for h in range(2):
            hsl = slice(h * HW_, (h + 1) * HW_)
            nc.sync.dma_start(dbc[:, hsl], bcBC[h][:])
            # dx = delta * xc (own), wide
            for g in range(NG):
                nc.vector.tensor_mul(dxw[g][h][:], delw[g][h][:],
                                     xcow[g][h][:])
            for n in range(N_):
                Bb = bbp.tile([128, HW_], bf16, tag="Bb")
                nc.sync.dma_start(
                    Bb[:], dbc[n:n + 1, hsl].broadcast_to((128, HW_)))
                Cb = bbp.tile([128, HW_], bf16, tag="Cb")
                cb_dma = nc.sync.dma_start(
                    Cb[:], dbc[N_ + n:N_ + n + 1, hsl].broadcast_to((128, HW_)))
                if blk == 0 and h == 1 and n == N_ - 1:
                    tc.chain_iter_dep("rbgate", cb_dma.ins)
                for g in range(NG):
                    k = n * NG + g
                    dA = dap.tile([128, HW_], bf16, tag="dA")
                    nc.scalar.activation(dA[:], delw[g][h][:], AF.Exp,
                                         scale=w['acol'][g][:, n:n + 1])
                    dBx = scp.tile([128, HW_], bf16, tag="dBx")
                    beng = nc.gpsimd if n >= 11 else nc.vector
                    beng.tensor_mul(dBx[:], dxw[g][h][:], Bb[:])
                    ht = hp.tile([128, HW_], bf16, tag="h")
                    eng = nc.vector if (k % 32) < _SCAN_DVE else nc.gpsimd
                    eng.tensor_tensor_scan(
                        ht[:], dA[:], dBx[:],
                        0.0 if h == 0 else carry[(g, n)][:],
                        op0=OPm, op1=OPa)
                    if h == 0:
                        cy = cyp.tile([128, 1], bf16, tag=f"cy{g}_{n}")
                        nc.gpsimd.tensor_copy(cy[:], ht[:, HW_ - 1:HW_])
                        carry[(g, n)] = cy
                    hC = scp.tile([128, HW_], bf16, tag="hC")
                    heng = nc.gpsimd if n < 9 else nc.vector
                    heng.tensor_mul(hC[:], ht[:], Cb[:])
                    for c2 in range(2):
                        if n == 0 and g == 0 and c2 == 0:
                            pacc = [[ps_acc.tile([128, TCk], f32,
                                                 tag=f"acc{gg}{cc}",
                                                 name=f"pac{blk}{h}_{gg}{cc}")
                                     for cc in range(2)] for gg in range(NG)]
                        nc.tensor.matmul(pacc[g][c2][:], ident[:],
                                         hC[:, ts(c2, TCk)],
                                         start=(n == 0), stop=(n == N_ - 1))
            for g in range(NG):
                acc = widep.tile([128, HW_], bf16, tag=f"acc{g}{h}",
                                 name=f"acc{blk}_{g}{h}")
                accs[h][g] = acc
                for c2 in range(2):
                    nc.scalar.copy(acc[:, ts(c2, TCk)], pacc[g][c2][:])

            # ---- post + out-proj for this half
            gz = []
            for g in range(NG):
                yg = accs[h][g]
                nc.vector.scalar_tensor_tensor(
                    yg[:], xcow[g][h][:], w['dvec'][g][:], yg[:],
                    OPm, OPa)
                nc.vector.tensor_mul(yg[:], yg[:], zw[g][h][:])
                gz.append(yg)
            for c2 in range(2):
                c = 2 * h + c2
                cs = ts(c, TCk)
                hs = ts(c2, TCk)
                for m in range(NXT):
                    pso = ps_out.tile([128, TCk], f32, tag="pso",
                                      name=f"pso{blk}_{c}_{m}")
                    for g in range(NG):
                        nc.tensor.matmul(pso[:], w['outw'][g][:, ts(m, 128)],
                                         gz[g][:, hs], start=(g == 0),
                                         stop=(g == 1))
                    if blk == 0:
                        nc.scalar.copy(partial[m][h][:, hs], pso[:])
                    else:
                        nc.vector.scalar_tensor_tensor(
                            preM[m][h][:, hs], xT[m][h][:, hs], alphav[:],
                            pso[:], OPm, OPa)
            if blk == 0:
                # issue this half's pair exchange (readback deferred so the
                # DMA queue keeps feeding the other half's scan)
                for m in range(NXT):
                    bi_dma = nc.sync.dma_start(
                        bounce_in[h][128 * m:128 * (m + 1), :],
                        partial[m][h][:])
                    if h == 1:
                        bounce_h1_dmas.append(bi_dma)
                if _USE_COLLECTIVE:
                    nc.gpsimd.collective_compute(
                        "AllGather", mybir.AluOpType.bypass,
                        replica_groups=replica_groups,
                        ins=[bounce_in[h][:].opt()],
                        outs=[bounce_out[h][:].opt()])
                else:
                    nc.sync.dma_start(bounce_out[h][0:DM, :], bounce_in[h][:])
                    nc.sync.dma_start(bounce_out[h][DM:2 * DM, :],
                                      bounce_in[h][:])
            else:
                # merge partial for this half
                for c2 in range(2):
                    c = 2 * h + c2
                    cs = ts(c, TCk)
                    hs = ts(c2, TCk)
                    for m in range(NXT):
                        ps = ps_out.tile([128, TCk], f32, tag="pso",
                                         name=f"mg{c}_{m}")
                        for j in range(2):
                            nc.tensor.matmul(ps[:], mergew[j][:, ts(m, 128)],
                                             preM[j][h][:, hs],
                                             start=(j == 0), stop=(j == 1))
                        op = pop.tile([128, TCk], f32, tag="op", bufs=2)
                        nc.scalar.copy(op[:], ps[:])
                        nc.sync.dma_start(outp[128 * m:128 * (m + 1), cs],
                                          op[:])
        if blk == 0:
            # h0 readback + residual adds, gated (chain dep) behind the h1
            # broadcast feed so its AllGather wait cannot starve the second
            # half's scan.  The h1 readback is deferred into block 1.
            for m in range(NXT):
                p0 = pop.tile([128, HW_], bf16, tag="peer0", bufs=2,
                              name=f"p0_0_{m}")
                p1 = pop.tile([128, HW_], bf16, tag="peer1", bufs=2,
                              name=f"p1_0_{m}")
                rb = nc.sync.dma_start(p0[:],
                                       bounce_out[0][128 * m:128 * (m + 1), :])
                tc.chain_iter_dep("rbgate", rb.ins)
                rb2 = nc.sync.dma_start(
                    p1[:], bounce_out[0][DM + 128 * m:DM + 128 * (m + 1), :])
                tc.chain_iter_dep("rbgate", rb2.ins)
                for c2 in range(2):
                    psx = ps_out.tile([128, TCk], f32, tag="pso",
                                      name=f"psx0_{m}_{c2}")
                    nc.tensor.matmul(psx[:], ident[:], p0[:, ts(c2, TCk)],
                                     start=True, stop=False)
                    nc.tensor.matmul(psx[:], ident[:], p1[:, ts(c2, TCk)],
                                     start=False, stop=False)
                    nc.tensor.matmul(psx[:], identf[:],
                                     xT[m][0][:, ts(c2, TCk)],
                                     start=False, stop=True)
                    nc.scalar.copy(xT_next[m][0][:, ts(c2, TCk)], psx[:])
            deferred_rb = (bounce_out, xT)
            xT = xT_next


# ---------------------------------------------------------------- entry

def kernel(**inputs) -> np.ndarray:
    from concourse.bass_utils import run_bass_kernel_spmd

    if 'nc' not in _BUILD_CACHE:
        _BUILD_CACHE['nc'] = _build_program()
    nc = _BUILD_CACHE['nc']

    in_maps = _host_inputs(inputs)
    res = run_bass_kernel_spmd(nc, in_maps, core_ids=list(range(8)))
    _BUILD_CACHE['last_res'] = res
    parts = [r['outp'] for r in res.results]   # [256, 2048] each

    out = np.zeros((B_, S_, DM), np.float32)
    for b in range(B_):
        acc = np.zeros((DM, S_), np.float32)
        for dh in range(2):
            acc += parts[0 * 4 + b * 2 + dh]               # fwd
            acc += parts[1 * 4 + b * 2 + dh][:, ::-1]      # bwd
        out[b] = acc.T
    return out
